# revision 23
# baseline (speedup 1.0000x reference)
"""Trainium2 Bass kernel for AudioAttentionMapGenerator.

Math (reference):
    sigma = exp(log_sigma); c = -0.5 / (sigma^2 + 1e-6)
    w_n   = attn_weights * mask
    map[b,h,w] = sum_n w_n * exp(c*((h-v_bn)^2 + (w-u_bn)^2))
    out = map / (max_hw(map) + 1e-6)

Key restructuring vs the previous STT-based kernel: the Gaussian exponent is
an affine function of the grid, so the host packs per-point coefficient rows
    a_u = -2c*u,  b_u = c*u^2 + ln(w),  a_v = -2c*v,  b_v = c*v^2,  c
and a constant grid matrix M = [grid|0, 1|0, 0|grid, 0|1, grid2|grid2].
One [10,448] DMA delivers both; a single f32r matmul per sample
(S^T [5x128] @ M [5x448]) produces the full exponent tile
[128 pts, 448 = x-block|y-block] in PSUM, and one Exp per sample turns it
into the weighted Gaussian factors (ln w folded into the x-block bias, so
the separable map matmul needs no extra weighting pass):
    map = Gy^T @ (w*Gx), two bf16 matmuls per sample (h chunked 128+96).

Per-sample normalization: stride-2-subsampled free-dim reduce_max (DVE; the
map's curvature scale is sigma=20px so a 2px grid undersamples the peak by
<0.1%), partition all-reduce (GPSIMD), reciprocal (DVE), then the scale is
split h-chunk-wise across ACT and DVE so each sample's scale finishes in one
chunk-time and both engines' pieces overlap.

Latency plumbing (the bulk of the win):
  - The input DMACopy is moved (post-build IR surgery) into block 0 BEFORE
    the preamble barrier: its 650ns SP-SEQ dispatch, 625ns HWDGE generation
    and 650ns DGE->DMA delay all run from t=0 instead of after the barrier.
  - The framework's four const-AP memsets (unused by this kernel) are
    deleted from block 0; they otherwise hold the preamble barrier (and
    with it every engine's start) hostage for ~440ns behind Pool.
  - Output leaves via a kv_writeback prepared early and fired by
    trigger_dma after the four scale pieces (staging buffer under two
    aliased SBUF names so the prep can run ~4us before the scales).
  - The Tile epilogue's two cross-engine barrier rounds after the final
    DMA-completion wait are stripped (the Pool queue still gates kernel end
    on kv_dma>=16; the barriers only re-synchronized already-idle engines).
  - PE warm-up matmuls tile the gaps so the exponent matmuls hit the mid
    p-state and the map matmuls hit the full-speed clock.
"""

import sys

import numpy as np

if "/opt/trn_rl_repo" not in sys.path:
    sys.path.insert(0, "/opt/trn_rl_repo")

B, N, H, W = 16, 128, 224, 224
NCORES = 8
BPC = B // NCORES  # samples per core
HP = 256  # padded rows per sample in DRAM (2 chunks of 128)
W2 = 2 * W  # 448: x-block | y-block

_CACHE = {}


def _build():
    if "nc" in _CACHE:
        return _CACHE["nc"]

    from contextlib import ExitStack

    import concourse.bass_isa as bass_isa
    import concourse.tile as tile
    from concourse import bacc, mybir
    from concourse.tile import add_dep_helper

    f32 = mybir.dt.float32
    f32r = mybir.dt.float32r
    bf16 = mybir.dt.bfloat16
    i32 = mybir.dt.int32
    AF = mybir.ActivationFunctionType
    AX = mybir.AxisListType

    nc = bacc.Bacc(
        "TRN2",
        target_bir_lowering=False,
        debug=False,
        enable_asserts=False,
        num_devices=NCORES,
    )
    # packed per-core input, one row-major DMA.  Matmul operands must share
    # base partition 0, so M and S live side by side in the free dim of one
    # [5, 896] tile: cols 0:448 = M = [grid|0, ones|0, 0|grid, 0|ones,
    # grid2|grid2]; cols 448:576 = S0 rows [a_u0, b_u0, a_v0, b_v0, c];
    # cols 576:704 = S1 rows.
    pkm = nc.dram_tensor("pkm", (5, 2 * W2), f32r, kind="ExternalInput").ap()
    # per-sample rows padded 224->256 so every 128-row writeback chunk stays
    # in-bounds; host discards rows 224:256 of each sample
    out = nc.dram_tensor("out", (BPC * HP, W), f32, kind="ExternalOutput")

    with ExitStack() as ctx:
        tc = ctx.enter_context(tile.TileContext(nc))
        consts = ctx.enter_context(tc.tile_pool(name="consts", bufs=1))
        psum = ctx.enter_context(tc.tile_pool(name="psum", bufs=1, space="PSUM"))

        # ---- input DMA (relocated to block 0 pre-barrier by surgery below).
        pkt = consts.tile([5, 2 * W2], f32r)
        nc.sync.dma_start(out=pkt, in_=pkm)

        # ---- output writeback descriptors prepared NOW, fired at the end.
        # out viewed as [batch=4 chunks, 128 rows, dho=1, 224]; staging is
        # [128 partitions, 4 chunks, 224] (chunk c partition p = row 128c+p).
        # Two aliased names for the same SBUF bytes: scales write st_w, the
        # prep's descriptors read st_r (Tile would otherwise WAR-pin the
        # scales to the in-flight DMA); real ordering is via trigger deps.
        st_w = nc.alloc_sbuf_tensor("st_w", (128, 2 * BPC, W), f32)
        st_r = nc.alloc_sbuf_tensor_at(
            "st_r", (128, 2 * BPC, W), f32, offset=nc.lookup_mloc(st_w).addr
        )
        st = st_w.ap()
        ctxi = consts.tile([128, 2 * BPC], i32)
        nc.gpsimd.memset(ctxi, 0)
        dma_sem = nc.alloc_semaphore("kv_dma")
        out4d = out.ap().rearrange("(c p) (d w) -> c p d w", p=128, d=1)
        i_kvprep = nc.gpsimd.kv_writeback(
            out4d,
            st_r.ap().rearrange("p (d b) w -> p d b w", d=1),
            ctxi[:, :],
            prepare_only=True,
            sem=dma_sem,
        )


        # Sample processing order is [1, 0]: sample1's exponent, Exp, coarse
        # map and full normalization chain all complete while sample0's Exp
        # is still running, so the two samples' reduce chains never contend.
        # PSUM ownership per engine (an ACT activation reading PSUM consumes
        # the accumulator, serializing any other toucher of that tile):
        #   ACT: eps0, eps1 (Exps), pmap1 (scale_s1), pm0c1 (scale_s0c1)
        #   DVE: cm0, cm1 (reduces), pm0c0 (scale_s0c0)
        eps, exps = [], []
        for b in range(BPC):
            ep = psum.tile([128, W2], f32, tag=f"eps{b}", name=f"eps{b}")
            eps.append(ep)
            exps.append(consts.tile([128, W2], bf16, tag=f"exp{b}", name=f"exp{b}"))
        pmap1 = psum.tile([128, 2, W], f32, tag="pmap1", name="pmap1")
        nc.vector.memset(pmap1[96:128, 1, :], 0.0)
        pmap0 = psum.tile([128, 2, W], f32, tag="pmap0", name="pmap0")
        nc.vector.memset(pmap0[96:128, 1, :], 0.0)

        # ---- exponent matmuls: ep[n, x|y] = S_b^T @ M  (f32r, 1 cyc/col) ----
        mov = pkt[0:5, 0:W2]
        i_mmexp = {}
        for b in (1, 0):
            stat = pkt[0:5, W2 + b * N : W2 + (b + 1) * N]
            i_mmexp[b] = nc.tensor.matmul(eps[b][:, :], stat, mov)

        # ---- Exps: one per sample, PSUM -> bf16 SBUF.  Sample0 (the late
        # chain) additionally gets a small coarse Exp over a stride-3
        # subsample of its exponent, emitted BEFORE its full Exp: its coarse
        # map and max chain start ~250ns earlier, while its full-map matmuls
        # (which have ~500ns of slack against the max chain) absorb the
        # 310ns Exp delay.
        CW = (W + 2) // 3  # 75
        i_exp = {}
        i_exp[1] = nc.scalar.activation(exps[1], eps[1][:, :], AF.Exp)
        exp0c = consts.tile([128, 2 * CW], bf16)
        i_exp0c = nc.scalar.activation(exp0c, eps[0][:, 0 : W2 : 3], AF.Exp)
        i_exp[0] = nc.scalar.activation(exps[0], eps[0][:, :], AF.Exp)

        # coarse maps for the max: stride-3 subsample in BOTH dims -> [75,75]
        # (peak deficit <= c*(1.5^2*2) ~ 0.56% relative, uniform)
        cms, i_cms = {}, {}
        cp1 = psum.tile([128, CW], f32, tag="cmap1", name="cmap1")
        i_cms[1] = nc.tensor.matmul(
            cp1[0:CW, :], exps[1][:, W : W2 : 3], exps[1][:, 0 : W : 3]
        )
        cms[1] = cp1
        cp0 = psum.tile([128, CW], f32, tag="cmap0", name="cmap0")
        i_cms[0] = nc.tensor.matmul(
            cp0[0:CW, :], exp0c[:, CW : 2 * CW], exp0c[:, 0:CW]
        )
        cms[0] = cp0

        # ---- map matmuls (bf16): chunks of 128/96 h-rows ----
        i_maps = [
            nc.tensor.matmul(pmap1[:, 0, :], exps[1][:, W : W + 128], exps[1][:, 0:W]),
            nc.tensor.matmul(pmap1[0:96, 1, :], exps[1][:, W + 128 : W2], exps[1][:, 0:W]),
            nc.tensor.matmul(pmap0[:, 0, :], exps[0][:, W : W + 128], exps[0][:, 0:W]),
            nc.tensor.matmul(pmap0[0:96, 1, :], exps[0][:, W + 128 : W2], exps[0][:, 0:W]),
        ]

        # ---- PE queue order ----
        peq = [
            i_mmexp[1],
            i_mmexp[0],
            i_cms[1],
            i_maps[0],
            i_maps[1],
            i_cms[0],
            i_maps[2],
            i_maps[3],
        ]
        for a, b_ in zip(peq[1:], peq[:-1]):
            add_dep_helper(a.ins, b_.ins, sync=False, reason="pe order")

        # ---- per-sample normalization (max from the coarse maps) ----
        mrows, malls, rss = {}, {}, {}
        i_red, i_rs = {}, {}
        for b in (1, 0):
            mrow = consts.tile([128, 1], f32, tag=f"mrow{b}", name=f"mrow{b}")
            nc.vector.memset(mrow[64:128, :], 0.0)
            i_red[b] = nc.vector.reduce_max(mrow[0:CW, :], cms[b][0:CW, :], axis=AX.XY)
            mrows[b] = mrow
        for b in (1, 0):
            mall = consts.tile([128, 1], f32, tag=f"mall{b}", name=f"mall{b}")
            nc.gpsimd.partition_all_reduce(
                mall, mrows[b], channels=128, reduce_op=bass_isa.ReduceOp.max
            )
            malls[b] = mall
        for b in (1, 0):
            rs = consts.tile([128, 1], f32, tag=f"rs{b}", name=f"rs{b}")
            i_rs[b] = nc.vector.reciprocal(rs, malls[b])
            rss[b] = rs
        # scales: sample1 (early chain) fully on ACT; sample0 (late chain)
        # fully on DVE — its reciprocal lands on the same engine just before
        i_scales = [
            nc.scalar.mul(st[:, 2:4, :], pmap1[:, :, :], rss[1][:, 0:1]),
            nc.vector.tensor_scalar_mul(st[:, 0:2, :], pmap0[:, :, :], rss[0][:, 0:1]),
        ]

        # DVE queue order: r1, r0, rs1, rs0, scale_s0
        dveq = [i_red[1], i_red[0], i_rs[1], i_rs[0], i_scales[1]]
        for a, b_ in zip(dveq[1:], dveq[:-1]):
            add_dep_helper(a.ins, b_.ins, sync=False, reason="dve order")
        # ACT queue order: exp1, exp0c, exp0, scale_s1
        actq = [i_exp[1], i_exp0c, i_exp[0], i_scales[0]]
        for a, b_ in zip(actq[1:], actq[:-1]):
            add_dep_helper(a.ins, b_.ins, sync=False, reason="act order")

        # ---- fire the prepared writeback ----
        trig = nc.gpsimd.trigger_dma(count=None)
        for s in i_scales:
            add_dep_helper(trig.ins, s.ins, sync=True, reason="st written")
        # pin the completion wait AFTER the trigger in the in-order Pool queue
        # (an SP-side wait on this sem crashes the device)
        wsem = nc.gpsimd.wait_ge(dma_sem, 16)
        add_dep_helper(wsem.ins, trig.ins, sync=False, reason="wait after fire")

    fn = nc.m.functions[0]
    ET = mybir.EngineType

    # Tile's epilogue waits on the prep's DMASW proc sem, which only the real
    # SWDGE hardware auto-bumps — drop them (kernel end is still gated on the
    # true DMA-completion sem via the Pool queue).
    for block in fn.blocks:
        for ins in block.instructions:
            si = ins.sync_info
            if si is None or not si.on_wait:
                continue
            if any(w.ant_name and w.ant_name.startswith("DMASW") for w in si.on_wait):
                si.on_wait = [
                    w
                    for w in si.on_wait
                    if not (w.ant_name and w.ant_name.startswith("DMASW"))
                ]

    # Fold standalone Pool event-sem waits into the trigger instruction
    for block in fn.blocks:
        insts = list(block.instructions)
        for idx, ins in enumerate(insts):
            if type(ins).__name__ != "InstTriggerDma" or ins.sync_info is None:
                continue
            j = idx - 1
            while j >= 0:
                p = insts[j]
                psi = p.sync_info
                if (
                    type(p).__name__ == "InstEventSemaphore"
                    and p.engine == ET.Pool
                    and psi is not None
                    and not psi.on_update
                    and psi.on_wait
                ):
                    ins.sync_info.on_wait = list(psi.on_wait) + list(
                        ins.sync_info.on_wait
                    )
                    psi.on_wait = []
                    j -= 1
                else:
                    break

    # ---- delete the framework const-AP memsets from block 0 (they gate the
    # preamble barrier behind ~440ns of Pool time).  Safety: only delete if
    # no other instruction references a const-* tensor.
    def _memrefs(ins):
        refs = []
        for o in list(getattr(ins, "outs", [])) + list(getattr(ins, "ins", [])):
            r = getattr(o, "memref", None)
            if r:
                refs.append(r)
        return refs

    used_consts = set()
    b0_const_memsets = []
    for bi, block in enumerate(fn.blocks):
        for ins in block.instructions:
            refs = [r for r in _memrefs(ins) if r.startswith("const-")]
            if not refs:
                continue
            if bi == 0 and type(ins).__name__ == "InstMemset":
                b0_const_memsets.append(ins)
            else:
                used_consts.update(refs)
    dead = [
        i for i in b0_const_memsets if not (set(_memrefs(i)) & used_consts)
    ]
    fn.blocks[0].instructions = [
        i for i in fn.blocks[0].instructions if i not in dead
    ]

    # ---- move the input DMACopy into block 0, before SP's preamble drain:
    # its 650ns SEQ dispatch + 625ns HWDGE generation + 650ns DGE delay then
    # run from t=0 instead of after the barrier.
    b0, b1 = fn.blocks[0], fn.blocks[1]
    dma_ins = None
    for ins in b1.instructions:
        if type(ins).__name__ == "InstDMACopy" and ins.engine == ET.SP:
            dma_ins = ins
            break
    if dma_ins is not None and (dma_ins.sync_info is None or not dma_ins.sync_info.on_wait):
        b1.instructions = [i for i in b1.instructions if i is not dma_ins]
        sp_drain_idx = next(
            i
            for i, ins in enumerate(b0.instructions)
            if type(ins).__name__ == "InstDrain" and ins.engine == ET.SP
        )
        b0.instructions = (
            b0.instructions[: sp_drain_idx + 1]
            + [dma_ins]
            + b0.instructions[sp_drain_idx + 1 :]
        )

    # ---- strip the epilogue's two cross-engine barrier rounds (block 2):
    # delete the barrier event-sems and un-wire the drains' barrier waits and
    # gather bumps.  Kernel end stays gated on kv_dma>=16 via the Pool queue.
    b2 = fn.blocks[2]
    pool_drains = [
        i
        for i in b2.instructions
        if type(i).__name__ == "InstDrain" and i.engine == ET.Pool
    ]
    kept = []
    for ins in b2.instructions:
        nm = getattr(ins, "name", "") or ""
        if type(ins).__name__ == "InstEventSemaphore" and nm.startswith("barrier_"):
            continue
        if ins in pool_drains:
            continue
        si = ins.sync_info
        if si is not None:
            if si.on_wait:
                si.on_wait = [
                    w
                    for w in si.on_wait
                    if not (w.ant_name and w.ant_name.startswith("barrier_"))
                ]
            if si.on_update:
                si.on_update = [
                    u
                    for u in si.on_update
                    if not (u.ant_name and u.ant_name.startswith("barrier_"))
                ]
        kept.append(ins)
    b2.instructions = kept

    nc.compile()
    _CACHE["nc"] = nc
    return nc


def kernel(pixel_coords, attn_weights, in_frame_mask, log_sigma, **kwargs):
    pc = np.asarray(pixel_coords, dtype=np.float64)  # (B, N, 2)
    aw = np.asarray(attn_weights, dtype=np.float64)
    mf = np.asarray(in_frame_mask).astype(np.float64)
    ls = float(np.asarray(log_sigma, dtype=np.float32))

    sig2 = np.exp(2.0 * ls)
    c = -0.5 / (sig2 + 1e-6)
    w = aw * mf
    lnw = np.log(np.maximum(w, 1e-20))  # clamp: exp(-46) == 0 for any pixel
    grid = np.arange(W, dtype=np.float64)

    nc = _build()
    from concourse.bass_utils import run_bass_kernel_spmd

    in_maps = []
    for i in range(NCORES):
        pkm = np.zeros((5, 2 * W2), dtype=np.float64)
        pkm[0, 0:W] = grid
        pkm[1, 0:W] = 1.0
        pkm[2, W:W2] = grid
        pkm[3, W:W2] = 1.0
        pkm[4, 0:W] = grid * grid
        pkm[4, W:W2] = grid * grid
        for b in range(BPC):
            s = BPC * i + b
            cs = slice(W2 + N * b, W2 + N * (b + 1))
            u = pc[s, :, 0]
            v = pc[s, :, 1]
            pkm[0, cs] = -2.0 * c * u
            pkm[1, cs] = c * u * u + lnw[s]
            pkm[2, cs] = -2.0 * c * v
            pkm[3, cs] = c * v * v
            pkm[4, cs] = c
        in_maps.append({"pkm": pkm.astype(np.float32)})
    res = run_bass_kernel_spmd(nc, in_maps, core_ids=list(range(NCORES)))
    return np.concatenate(
        [r["out"].reshape(BPC, HP, W)[:, :H, :] for r in res.results], axis=0
    )


# revision 25
# speedup vs baseline: 1.0333x; 1.0333x over previous
"""Trainium2 Bass kernel for AudioAttentionMapGenerator.

Math (reference):
    sigma = exp(log_sigma); c = -0.5 / (sigma^2 + 1e-6)
    w_n   = attn_weights * mask
    map[b,h,w] = sum_n w_n * exp(c*((h-v_bn)^2 + (w-u_bn)^2))
    out = map / (max_hw(map) + 1e-6)

Key restructuring vs the previous STT-based kernel: the Gaussian exponent is
an affine function of the grid, so the host packs per-point coefficient rows
    a_u = -2c*u,  b_u = c*u^2 + ln(w),  a_v = -2c*v,  b_v = c*v^2,  c
and a constant grid matrix M = [grid|0, 1|0, 0|grid, 0|1, grid2|grid2].
One [10,448] DMA delivers both; a single f32r matmul per sample
(S^T [5x128] @ M [5x448]) produces the full exponent tile
[128 pts, 448 = x-block|y-block] in PSUM, and one Exp per sample turns it
into the weighted Gaussian factors (ln w folded into the x-block bias, so
the separable map matmul needs no extra weighting pass):
    map = Gy^T @ (w*Gx), two bf16 matmuls per sample (h chunked 128+96).

Per-sample normalization: stride-2-subsampled free-dim reduce_max (DVE; the
map's curvature scale is sigma=20px so a 2px grid undersamples the peak by
<0.1%), partition all-reduce (GPSIMD), reciprocal (DVE), then the scale is
split h-chunk-wise across ACT and DVE so each sample's scale finishes in one
chunk-time and both engines' pieces overlap.

Latency plumbing (the bulk of the win):
  - The input DMACopy is moved (post-build IR surgery) into block 0 BEFORE
    the preamble barrier: its 650ns SP-SEQ dispatch, 625ns HWDGE generation
    and 650ns DGE->DMA delay all run from t=0 instead of after the barrier.
  - The framework's four const-AP memsets (unused by this kernel) are
    deleted from block 0; they otherwise hold the preamble barrier (and
    with it every engine's start) hostage for ~440ns behind Pool.
  - Output leaves via a kv_writeback prepared early and fired by
    trigger_dma after the four scale pieces (staging buffer under two
    aliased SBUF names so the prep can run ~4us before the scales).
  - The Tile epilogue's two cross-engine barrier rounds after the final
    DMA-completion wait are stripped (the Pool queue still gates kernel end
    on kv_dma>=16; the barriers only re-synchronized already-idle engines).
  - PE warm-up matmuls tile the gaps so the exponent matmuls hit the mid
    p-state and the map matmuls hit the full-speed clock.
"""

import sys

import numpy as np

if "/opt/trn_rl_repo" not in sys.path:
    sys.path.insert(0, "/opt/trn_rl_repo")

B, N, H, W = 16, 128, 224, 224
NCORES = 8
BPC = B // NCORES  # samples per core
HP = 256  # padded rows per sample in DRAM (2 chunks of 128)
W2 = 2 * W  # 448: x-block | y-block

_CACHE = {}


def _build():
    if "nc" in _CACHE:
        return _CACHE["nc"]

    from contextlib import ExitStack

    import concourse.bass_isa as bass_isa
    import concourse.tile as tile
    from concourse import bacc, mybir
    from concourse.tile import add_dep_helper

    f32 = mybir.dt.float32
    f32r = mybir.dt.float32r
    bf16 = mybir.dt.bfloat16
    i32 = mybir.dt.int32
    AF = mybir.ActivationFunctionType
    AX = mybir.AxisListType

    nc = bacc.Bacc(
        "TRN2",
        target_bir_lowering=False,
        debug=False,
        enable_asserts=False,
        num_devices=NCORES,
    )
    # packed per-core input, one row-major DMA.  Matmul operands must share
    # base partition 0, so M and S live side by side in the free dim of one
    # [5, 896] tile: cols 0:448 = M = [grid|0, ones|0, 0|grid, 0|ones,
    # grid2|grid2]; cols 448:576 = S0 rows [a_u0, b_u0, a_v0, b_v0, c];
    # cols 576:704 = S1 rows.
    pkm = nc.dram_tensor("pkm", (5, 2 * W2), f32r, kind="ExternalInput").ap()
    # per-sample rows padded 224->256 so every 128-row writeback chunk stays
    # in-bounds; host discards rows 224:256 of each sample
    out = nc.dram_tensor("out", (BPC * HP, W), f32, kind="ExternalOutput")

    with ExitStack() as ctx:
        tc = ctx.enter_context(tile.TileContext(nc))
        consts = ctx.enter_context(tc.tile_pool(name="consts", bufs=1))
        psum = ctx.enter_context(tc.tile_pool(name="psum", bufs=1, space="PSUM"))

        # ---- input DMA (relocated to block 0 pre-barrier by surgery below).
        pkt = consts.tile([5, 2 * W2], f32r)
        nc.sync.dma_start(out=pkt, in_=pkm)

        # ---- output writeback descriptors prepared NOW, fired at the end.
        # out viewed as [batch=4 chunks, 128 rows, dho=1, 224]; staging is
        # [128 partitions, 4 chunks, 224] (chunk c partition p = row 128c+p).
        # Two aliased names for the same SBUF bytes: scales write st_w, the
        # prep's descriptors read st_r (Tile would otherwise WAR-pin the
        # scales to the in-flight DMA); real ordering is via trigger deps.
        st_w = nc.alloc_sbuf_tensor("st_w", (128, 2 * BPC, W), f32)
        st_r = nc.alloc_sbuf_tensor_at(
            "st_r", (128, 2 * BPC, W), f32, offset=nc.lookup_mloc(st_w).addr
        )
        st = st_w.ap()
        ctxi = consts.tile([128, 2 * BPC], i32)
        nc.gpsimd.memset(ctxi, 0)
        dma_sem = nc.alloc_semaphore("kv_dma")
        out4d = out.ap().rearrange("(c p) (d w) -> c p d w", p=128, d=1)
        i_kvprep = nc.gpsimd.kv_writeback(
            out4d,
            st_r.ap().rearrange("p (d b) w -> p d b w", d=1),
            ctxi[:, :],
            prepare_only=True,
            sem=dma_sem,
        )


        # Sample processing order is [1, 0]: sample1's exponent, Exp, coarse
        # map and full normalization chain all complete while sample0's Exp
        # is still running, so the two samples' reduce chains never contend.
        # PSUM ownership per engine (an ACT activation reading PSUM consumes
        # the accumulator, serializing any other toucher of that tile):
        #   ACT: eps0, eps1 (Exps), pmap1 (scale_s1), pm0c1 (scale_s0c1)
        #   DVE: cm0, cm1 (reduces), pm0c0 (scale_s0c0)
        eps, exps = [], []
        for b in range(BPC):
            ep = psum.tile([128, W2], f32, tag=f"eps{b}", name=f"eps{b}")
            eps.append(ep)
            exps.append(consts.tile([128, W2], bf16, tag=f"exp{b}", name=f"exp{b}"))
        pmap1 = psum.tile([128, 2, W], f32, tag="pmap1", name="pmap1")
        nc.vector.memset(pmap1[96:128, 1, :], 0.0)
        pmap0 = psum.tile([128, 2, W], f32, tag="pmap0", name="pmap0")
        nc.vector.memset(pmap0[96:128, 1, :], 0.0)

        # ---- exponent matmuls: ep[n, x|y] = S_b^T @ M  (f32r, 1 cyc/col) ----
        mov = pkt[0:5, 0:W2]
        i_mmexp = {}
        for b in (1, 0):
            stat = pkt[0:5, W2 + b * N : W2 + (b + 1) * N]
            i_mmexp[b] = nc.tensor.matmul(eps[b][:, :], stat, mov)

        # ---- Exps: one per sample, PSUM -> bf16 SBUF.  Sample0 (the late
        # chain) additionally gets a small coarse Exp over a stride-3
        # subsample of its exponent, emitted BEFORE its full Exp: its coarse
        # map and max chain start ~250ns earlier, while its full-map matmuls
        # (which have ~500ns of slack against the max chain) absorb the
        # 310ns Exp delay.
        CW = (W + 2) // 3  # 75
        i_exp = {}
        i_exp[1] = nc.scalar.activation(exps[1], eps[1][:, :], AF.Exp)
        exp0c = consts.tile([128, 2 * CW], bf16)
        i_exp0c = nc.scalar.activation(exp0c, eps[0][:, 0 : W2 : 3], AF.Exp)
        i_exp[0] = nc.scalar.activation(exps[0], eps[0][:, :], AF.Exp)

        # coarse maps for the max: stride-3 subsample in BOTH dims -> [75,75]
        # (peak deficit <= c*(1.5^2*2) ~ 0.56% relative, uniform)
        cms, i_cms = {}, {}
        cp1 = psum.tile([128, CW], f32, tag="cmap1", name="cmap1")
        i_cms[1] = nc.tensor.matmul(
            cp1[0:CW, :], exps[1][:, W : W2 : 3], exps[1][:, 0 : W : 3]
        )
        cms[1] = cp1
        cp0 = psum.tile([128, CW], f32, tag="cmap0", name="cmap0")
        i_cms[0] = nc.tensor.matmul(
            cp0[0:CW, :], exp0c[:, CW : 2 * CW], exp0c[:, 0:CW]
        )
        cms[0] = cp0

        # ---- map matmuls (bf16): chunks of 128/96 h-rows ----
        i_maps = [
            nc.tensor.matmul(pmap1[:, 0, :], exps[1][:, W : W + 128], exps[1][:, 0:W]),
            nc.tensor.matmul(pmap1[0:96, 1, :], exps[1][:, W + 128 : W2], exps[1][:, 0:W]),
            nc.tensor.matmul(pmap0[:, 0, :], exps[0][:, W : W + 128], exps[0][:, 0:W]),
            nc.tensor.matmul(pmap0[0:96, 1, :], exps[0][:, W + 128 : W2], exps[0][:, 0:W]),
        ]

        # ---- PE queue order ----
        peq = [
            i_mmexp[1],
            i_mmexp[0],
            i_cms[1],
            i_maps[0],
            i_maps[1],
            i_cms[0],
            i_maps[2],
            i_maps[3],
        ]
        for a, b_ in zip(peq[1:], peq[:-1]):
            add_dep_helper(a.ins, b_.ins, sync=False, reason="pe order")

        # ---- per-sample normalization (max from the coarse maps) ----
        mrows, malls, rss = {}, {}, {}
        i_red, i_rs = {}, {}
        for b in (1, 0):
            mrow = consts.tile([128, 1], f32, tag=f"mrow{b}", name=f"mrow{b}")
            nc.vector.memset(mrow[64:128, :], 0.0)
            i_red[b] = nc.vector.reduce_max(mrow[0:CW, :], cms[b][0:CW, :], axis=AX.XY)
            mrows[b] = mrow
        for b in (1, 0):
            mall = consts.tile([128, 1], f32, tag=f"mall{b}", name=f"mall{b}")
            nc.gpsimd.partition_all_reduce(
                mall, mrows[b], channels=128, reduce_op=bass_isa.ReduceOp.max
            )
            malls[b] = mall
        for b in (1, 0):
            rs = consts.tile([128, 1], f32, tag=f"rs{b}", name=f"rs{b}")
            i_rs[b] = nc.vector.reciprocal(rs, malls[b])
            rss[b] = rs
        # scales: sample1 (early chain) fully on ACT; sample0 (late chain)
        # fully on DVE — its reciprocal lands on the same engine just before
        i_scales = [
            nc.scalar.mul(st[:, 2:4, :], pmap1[:, :, :], rss[1][:, 0:1]),
            nc.vector.tensor_scalar_mul(st[:, 0:2, :], pmap0[:, :, :], rss[0][:, 0:1]),
        ]

        # DVE queue order: r1, r0, rs1, rs0, scale_s0
        dveq = [i_red[1], i_red[0], i_rs[1], i_rs[0], i_scales[1]]
        for a, b_ in zip(dveq[1:], dveq[:-1]):
            add_dep_helper(a.ins, b_.ins, sync=False, reason="dve order")
        # ACT queue order: exp1, exp0c, exp0, scale_s1
        actq = [i_exp[1], i_exp0c, i_exp[0], i_scales[0]]
        for a, b_ in zip(actq[1:], actq[:-1]):
            add_dep_helper(a.ins, b_.ins, sync=False, reason="act order")

        # ---- fire the prepared writeback ----
        trig = nc.gpsimd.trigger_dma(count=None)
        for s in i_scales:
            add_dep_helper(trig.ins, s.ins, sync=True, reason="st written")
        # pin the completion wait AFTER the trigger in the in-order Pool queue
        # (an SP-side wait on this sem crashes the device)
        wsem = nc.gpsimd.wait_ge(dma_sem, 16)
        add_dep_helper(wsem.ins, trig.ins, sync=False, reason="wait after fire")

    fn = nc.m.functions[0]
    ET = mybir.EngineType

    # Same-engine proc-sem waits on ENGINE-executed compute ops are
    # redundant (the engine runs its queue in order) but cost ~SEM_DELAY
    # when the predecessor has only just finished; strip them.  Sequencer-
    # only instructions (triggers, event sems, waits) genuinely need them —
    # the SEQ runs ahead of the engine — so only compute ops are touched.
    _eng_prefix = {
        ET.Pool: "Pool_",
        ET.Activation: "Activation_",
        ET.DVE: "DVE_",
        ET.PE: "PE_",
    }
    _strippable = {
        "InstActivation",
        "InstTensorScalarPtr",
        "InstTensorReduce",
        "InstReciprocal",
        "InstMatmult",
        "InstMemset",
    }
    for block in fn.blocks:
        for ins in block.instructions:
            si = ins.sync_info
            pref = _eng_prefix.get(ins.engine)
            if (
                si is None
                or not si.on_wait
                or pref is None
                or type(ins).__name__ not in _strippable
            ):
                continue
            kept_w = [
                w
                for w in si.on_wait
                if not (w.ant_name and w.ant_name.startswith(pref))
            ]
            if len(kept_w) != len(si.on_wait):
                si.on_wait = kept_w

    # Tile's epilogue waits on the prep's DMASW proc sem, which only the real
    # SWDGE hardware auto-bumps — drop them (kernel end is still gated on the
    # true DMA-completion sem via the Pool queue).
    for block in fn.blocks:
        for ins in block.instructions:
            si = ins.sync_info
            if si is None or not si.on_wait:
                continue
            if any(w.ant_name and w.ant_name.startswith("DMASW") for w in si.on_wait):
                si.on_wait = [
                    w
                    for w in si.on_wait
                    if not (w.ant_name and w.ant_name.startswith("DMASW"))
                ]

    # Fold standalone Pool event-sem waits into the trigger instruction
    for block in fn.blocks:
        insts = list(block.instructions)
        for idx, ins in enumerate(insts):
            if type(ins).__name__ != "InstTriggerDma" or ins.sync_info is None:
                continue
            j = idx - 1
            while j >= 0:
                p = insts[j]
                psi = p.sync_info
                if (
                    type(p).__name__ == "InstEventSemaphore"
                    and p.engine == ET.Pool
                    and psi is not None
                    and not psi.on_update
                    and psi.on_wait
                ):
                    ins.sync_info.on_wait = list(psi.on_wait) + list(
                        ins.sync_info.on_wait
                    )
                    psi.on_wait = []
                    j -= 1
                else:
                    break

    # ---- delete the framework const-AP memsets from block 0 (they gate the
    # preamble barrier behind ~440ns of Pool time).  Safety: only delete if
    # no other instruction references a const-* tensor.
    def _memrefs(ins):
        refs = []
        for o in list(getattr(ins, "outs", [])) + list(getattr(ins, "ins", [])):
            r = getattr(o, "memref", None)
            if r:
                refs.append(r)
        return refs

    used_consts = set()
    b0_const_memsets = []
    for bi, block in enumerate(fn.blocks):
        for ins in block.instructions:
            refs = [r for r in _memrefs(ins) if r.startswith("const-")]
            if not refs:
                continue
            if bi == 0 and type(ins).__name__ == "InstMemset":
                b0_const_memsets.append(ins)
            else:
                used_consts.update(refs)
    dead = [
        i for i in b0_const_memsets if not (set(_memrefs(i)) & used_consts)
    ]
    fn.blocks[0].instructions = [
        i for i in fn.blocks[0].instructions if i not in dead
    ]

    # ---- move the input DMACopy into block 0, before SP's preamble drain:
    # its 650ns SEQ dispatch + 625ns HWDGE generation + 650ns DGE delay then
    # run from t=0 instead of after the barrier.
    b0, b1 = fn.blocks[0], fn.blocks[1]
    dma_ins = None
    for ins in b1.instructions:
        if type(ins).__name__ == "InstDMACopy" and ins.engine == ET.SP:
            dma_ins = ins
            break
    if dma_ins is not None and (dma_ins.sync_info is None or not dma_ins.sync_info.on_wait):
        b1.instructions = [i for i in b1.instructions if i is not dma_ins]
        sp_drain_idx = next(
            i
            for i, ins in enumerate(b0.instructions)
            if type(ins).__name__ == "InstDrain" and ins.engine == ET.SP
        )
        b0.instructions = (
            b0.instructions[: sp_drain_idx + 1]
            + [dma_ins]
            + b0.instructions[sp_drain_idx + 1 :]
        )

    # ---- strip the epilogue's two cross-engine barrier rounds (block 2):
    # delete the barrier event-sems and un-wire the drains' barrier waits and
    # gather bumps.  Kernel end stays gated on kv_dma>=16 via the Pool queue.
    b2 = fn.blocks[2]
    pool_drains = [
        i
        for i in b2.instructions
        if type(i).__name__ == "InstDrain" and i.engine == ET.Pool
    ]
    kept = []
    for ins in b2.instructions:
        nm = getattr(ins, "name", "") or ""
        if type(ins).__name__ == "InstEventSemaphore" and nm.startswith("barrier_"):
            continue
        if ins in pool_drains:
            continue
        si = ins.sync_info
        if si is not None:
            if si.on_wait:
                si.on_wait = [
                    w
                    for w in si.on_wait
                    if not (w.ant_name and w.ant_name.startswith("barrier_"))
                ]
            if si.on_update:
                si.on_update = [
                    u
                    for u in si.on_update
                    if not (u.ant_name and u.ant_name.startswith("barrier_"))
                ]
        kept.append(ins)
    b2.instructions = kept

    nc.compile()
    _CACHE["nc"] = nc
    return nc


def kernel(pixel_coords, attn_weights, in_frame_mask, log_sigma, **kwargs):
    pc = np.asarray(pixel_coords, dtype=np.float64)  # (B, N, 2)
    aw = np.asarray(attn_weights, dtype=np.float64)
    mf = np.asarray(in_frame_mask).astype(np.float64)
    ls = float(np.asarray(log_sigma, dtype=np.float32))

    sig2 = np.exp(2.0 * ls)
    c = -0.5 / (sig2 + 1e-6)
    w = aw * mf
    lnw = np.log(np.maximum(w, 1e-20))  # clamp: exp(-46) == 0 for any pixel
    grid = np.arange(W, dtype=np.float64)

    nc = _build()
    from concourse.bass_utils import run_bass_kernel_spmd

    in_maps = []
    for i in range(NCORES):
        pkm = np.zeros((5, 2 * W2), dtype=np.float64)
        pkm[0, 0:W] = grid
        pkm[1, 0:W] = 1.0
        pkm[2, W:W2] = grid
        pkm[3, W:W2] = 1.0
        pkm[4, 0:W] = grid * grid
        pkm[4, W:W2] = grid * grid
        for b in range(BPC):
            s = BPC * i + b
            cs = slice(W2 + N * b, W2 + N * (b + 1))
            u = pc[s, :, 0]
            v = pc[s, :, 1]
            pkm[0, cs] = -2.0 * c * u
            pkm[1, cs] = c * u * u + lnw[s]
            pkm[2, cs] = -2.0 * c * v
            pkm[3, cs] = c * v * v
            pkm[4, cs] = c
        in_maps.append({"pkm": pkm.astype(np.float32)})
    res = run_bass_kernel_spmd(nc, in_maps, core_ids=list(range(NCORES)))
    return np.concatenate(
        [r["out"].reshape(BPC, HP, W)[:, :H, :] for r in res.results], axis=0
    )


# revision 26
# speedup vs baseline: 1.0390x; 1.0055x over previous
"""Trainium2 Bass kernel for AudioAttentionMapGenerator.

Math (reference):
    sigma = exp(log_sigma); c = -0.5 / (sigma^2 + 1e-6)
    w_n   = attn_weights * mask
    map[b,h,w] = sum_n w_n * exp(c*((h-v_bn)^2 + (w-u_bn)^2))
    out = map / (max_hw(map) + 1e-6)

Key restructuring vs the previous STT-based kernel: the Gaussian exponent is
an affine function of the grid, so the host packs per-point coefficient rows
    a_u = -2c*u,  b_u = c*u^2 + ln(w),  a_v = -2c*v,  b_v = c*v^2,  c
and a constant grid matrix M = [grid|0, 1|0, 0|grid, 0|1, grid2|grid2].
One [10,448] DMA delivers both; a single f32r matmul per sample
(S^T [5x128] @ M [5x448]) produces the full exponent tile
[128 pts, 448 = x-block|y-block] in PSUM, and one Exp per sample turns it
into the weighted Gaussian factors (ln w folded into the x-block bias, so
the separable map matmul needs no extra weighting pass):
    map = Gy^T @ (w*Gx), two bf16 matmuls per sample (h chunked 128+96).

Per-sample normalization: stride-2-subsampled free-dim reduce_max (DVE; the
map's curvature scale is sigma=20px so a 2px grid undersamples the peak by
<0.1%), partition all-reduce (GPSIMD), reciprocal (DVE), then the scale is
split h-chunk-wise across ACT and DVE so each sample's scale finishes in one
chunk-time and both engines' pieces overlap.

Latency plumbing (the bulk of the win):
  - The input DMACopy is moved (post-build IR surgery) into block 0 BEFORE
    the preamble barrier: its 650ns SP-SEQ dispatch, 625ns HWDGE generation
    and 650ns DGE->DMA delay all run from t=0 instead of after the barrier.
  - The framework's four const-AP memsets (unused by this kernel) are
    deleted from block 0; they otherwise hold the preamble barrier (and
    with it every engine's start) hostage for ~440ns behind Pool.
  - Output leaves via a kv_writeback prepared early and fired by
    trigger_dma after the four scale pieces (staging buffer under two
    aliased SBUF names so the prep can run ~4us before the scales).
  - The Tile epilogue's two cross-engine barrier rounds after the final
    DMA-completion wait are stripped (the Pool queue still gates kernel end
    on kv_dma>=16; the barriers only re-synchronized already-idle engines).
  - PE warm-up matmuls tile the gaps so the exponent matmuls hit the mid
    p-state and the map matmuls hit the full-speed clock.
"""

import sys

import numpy as np

if "/opt/trn_rl_repo" not in sys.path:
    sys.path.insert(0, "/opt/trn_rl_repo")

B, N, H, W = 16, 128, 224, 224
NCORES = 8
BPC = B // NCORES  # samples per core
HP = 256  # padded rows per sample in DRAM (2 chunks of 128)
W2 = 2 * W  # 448: x-block | y-block

_CACHE = {}


def _build():
    if "nc" in _CACHE:
        return _CACHE["nc"]

    from contextlib import ExitStack

    import concourse.bass_isa as bass_isa
    import concourse.tile as tile
    from concourse import bacc, mybir
    from concourse.tile import add_dep_helper

    f32 = mybir.dt.float32
    f32r = mybir.dt.float32r
    bf16 = mybir.dt.bfloat16
    i32 = mybir.dt.int32
    AF = mybir.ActivationFunctionType
    AX = mybir.AxisListType

    nc = bacc.Bacc(
        "TRN2",
        target_bir_lowering=False,
        debug=False,
        enable_asserts=False,
        num_devices=NCORES,
    )
    # packed per-core input, one row-major DMA.  Matmul operands must share
    # base partition 0, so M and S live side by side in the free dim of one
    # [5, 896] tile: cols 0:448 = M = [grid|0, ones|0, 0|grid, 0|ones,
    # grid2|grid2]; cols 448:576 = S0 rows [a_u0, b_u0, a_v0, b_v0, c];
    # cols 576:704 = S1 rows.
    pkm = nc.dram_tensor("pkm", (5, 2 * W2), f32r, kind="ExternalInput").ap()
    # per-sample rows padded 224->256 so every 128-row writeback chunk stays
    # in-bounds; host discards rows 224:256 of each sample
    out = nc.dram_tensor("out", (BPC * HP, W), f32, kind="ExternalOutput")

    with ExitStack() as ctx:
        tc = ctx.enter_context(tile.TileContext(nc))
        consts = ctx.enter_context(tc.tile_pool(name="consts", bufs=1))
        psum = ctx.enter_context(tc.tile_pool(name="psum", bufs=1, space="PSUM"))

        # ---- input DMA (relocated to block 0 pre-barrier by surgery below).
        pkt = consts.tile([5, 2 * W2], f32r)
        nc.sync.dma_start(out=pkt, in_=pkm)

        # ---- output writeback descriptors prepared NOW, fired at the end.
        # out viewed as [batch=4 chunks, 128 rows, dho=1, 224]; staging is
        # [128 partitions, 4 chunks, 224] (chunk c partition p = row 128c+p).
        # Two aliased names for the same SBUF bytes: scales write st_w, the
        # prep's descriptors read st_r (Tile would otherwise WAR-pin the
        # scales to the in-flight DMA); real ordering is via trigger deps.
        st_w = nc.alloc_sbuf_tensor("st_w", (128, 2 * BPC, W), f32)
        st_r = nc.alloc_sbuf_tensor_at(
            "st_r", (128, 2 * BPC, W), f32, offset=nc.lookup_mloc(st_w).addr
        )
        st = st_w.ap()
        ctxi = consts.tile([128, 2 * BPC], i32)
        nc.gpsimd.memset(ctxi, 0)
        dma_sem = nc.alloc_semaphore("kv_dma")
        out4d = out.ap().rearrange("(c p) (d w) -> c p d w", p=128, d=1)
        i_kvprep = nc.gpsimd.kv_writeback(
            out4d,
            st_r.ap().rearrange("p (d b) w -> p d b w", d=1),
            ctxi[:, :],
            prepare_only=True,
            sem=dma_sem,
        )


        # Sample processing order is [1, 0]: sample1's exponent, Exp, coarse
        # map and full normalization chain all complete while sample0's Exp
        # is still running, so the two samples' reduce chains never contend.
        # PSUM ownership per engine (an ACT activation reading PSUM consumes
        # the accumulator, serializing any other toucher of that tile):
        #   ACT: eps0, eps1 (Exps), pmap1 (scale_s1), pm0c1 (scale_s0c1)
        #   DVE: cm0, cm1 (reduces), pm0c0 (scale_s0c0)
        eps, exps = [], []
        for b in range(BPC):
            ep = psum.tile([128, W2], f32, tag=f"eps{b}", name=f"eps{b}")
            eps.append(ep)
            exps.append(consts.tile([128, W2], bf16, tag=f"exp{b}", name=f"exp{b}"))
        pmap1 = psum.tile([128, 2, W], f32, tag="pmap1", name="pmap1")
        nc.vector.memset(pmap1[96:128, 1, :], 0.0)
        pmap0 = psum.tile([128, 2, W], f32, tag="pmap0", name="pmap0")
        nc.vector.memset(pmap0[96:128, 1, :], 0.0)

        # ---- exponent matmuls: ep[n, x|y] = S_b^T @ M  (f32r, 1 cyc/col) ----
        mov = pkt[0:5, 0:W2]
        i_mmexp = {}
        for b in (1, 0):
            stat = pkt[0:5, W2 + b * N : W2 + (b + 1) * N]
            i_mmexp[b] = nc.tensor.matmul(eps[b][:, :], stat, mov)

        # ---- Exps: one per sample, PSUM -> bf16 SBUF.  Sample0 (the late
        # chain) additionally gets a small coarse Exp over a stride-3
        # subsample of its exponent, emitted BEFORE its full Exp: its coarse
        # map and max chain start ~250ns earlier, while its full-map matmuls
        # (which have ~500ns of slack against the max chain) absorb the
        # 310ns Exp delay.
        CW = (W + 2) // 3  # 75
        i_exp = {}
        i_exp[1] = nc.scalar.activation(exps[1], eps[1][:, :], AF.Exp)
        exp0c = consts.tile([128, 2 * CW], bf16)
        i_exp0c = nc.scalar.activation(exp0c, eps[0][:, 0 : W2 : 3], AF.Exp)
        i_exp[0] = nc.scalar.activation(exps[0], eps[0][:, :], AF.Exp)

        # coarse maps for the max: stride-3 subsample in BOTH dims -> [75,75]
        # (peak deficit <= c*(1.5^2*2) ~ 0.56% relative, uniform)
        cms, i_cms = {}, {}
        cp1 = psum.tile([128, CW], f32, tag="cmap1", name="cmap1")
        i_cms[1] = nc.tensor.matmul(
            cp1[0:CW, :], exps[1][:, W : W2 : 3], exps[1][:, 0 : W : 3]
        )
        cms[1] = cp1
        cp0 = psum.tile([128, CW], f32, tag="cmap0", name="cmap0")
        i_cms[0] = nc.tensor.matmul(
            cp0[0:CW, :], exp0c[:, CW : 2 * CW], exp0c[:, 0:CW]
        )
        cms[0] = cp0

        # ---- map matmuls (bf16): chunks of 128/96 h-rows ----
        i_maps = [
            nc.tensor.matmul(pmap1[:, 0, :], exps[1][:, W : W + 128], exps[1][:, 0:W]),
            nc.tensor.matmul(pmap1[0:96, 1, :], exps[1][:, W + 128 : W2], exps[1][:, 0:W]),
            nc.tensor.matmul(pmap0[:, 0, :], exps[0][:, W : W + 128], exps[0][:, 0:W]),
            nc.tensor.matmul(pmap0[0:96, 1, :], exps[0][:, W + 128 : W2], exps[0][:, 0:W]),
        ]

        # ---- PE queue order ----
        peq = [
            i_mmexp[1],
            i_mmexp[0],
            i_cms[1],
            i_maps[0],
            i_maps[1],
            i_cms[0],
            i_maps[2],
            i_maps[3],
        ]
        for a, b_ in zip(peq[1:], peq[:-1]):
            add_dep_helper(a.ins, b_.ins, sync=False, reason="pe order")

        # ---- per-sample normalization (max from the coarse maps) ----
        mrows, malls, rss = {}, {}, {}
        i_red, i_rs = {}, {}
        for b in (1, 0):
            mrow = consts.tile([128, 1], f32, tag=f"mrow{b}", name=f"mrow{b}")
            nc.vector.memset(mrow[64:128, :], 0.0)
            i_red[b] = nc.vector.reduce_max(mrow[0:CW, :], cms[b][0:CW, :], axis=AX.XY)
            mrows[b] = mrow
        for b in (1, 0):
            mall = consts.tile([128, 1], f32, tag=f"mall{b}", name=f"mall{b}")
            nc.gpsimd.partition_all_reduce(
                mall, mrows[b], channels=128, reduce_op=bass_isa.ReduceOp.max
            )
            malls[b] = mall
        for b in (1, 0):
            rs = consts.tile([128, 1], f32, tag=f"rs{b}", name=f"rs{b}")
            i_rs[b] = nc.vector.reciprocal(rs, malls[b])
            rss[b] = rs
        # scales: sample1 (early chain) on DVE right after its reciprocal;
        # sample0 (late, maps-gated) on ACT which is free once Exp0 retires
        i_scales = [
            nc.vector.tensor_scalar_mul(st[:, 2:4, :], pmap1[:, :, :], rss[1][:, 0:1]),
            nc.scalar.mul(st[:, 0:2, :], pmap0[:, :, :], rss[0][:, 0:1]),
        ]

        # DVE queue order: r1, r0, rs1, rs0, scale_s1
        dveq = [i_red[1], i_red[0], i_rs[1], i_rs[0], i_scales[0]]
        for a, b_ in zip(dveq[1:], dveq[:-1]):
            add_dep_helper(a.ins, b_.ins, sync=False, reason="dve order")
        # ACT queue order: exp1, exp0c, exp0, scale_s0
        actq = [i_exp[1], i_exp0c, i_exp[0], i_scales[1]]
        for a, b_ in zip(actq[1:], actq[:-1]):
            add_dep_helper(a.ins, b_.ins, sync=False, reason="act order")

        # ---- fire the prepared writeback ----
        trig = nc.gpsimd.trigger_dma(count=None)
        for s in i_scales:
            add_dep_helper(trig.ins, s.ins, sync=True, reason="st written")
        # pin the completion wait AFTER the trigger in the in-order Pool queue
        # (an SP-side wait on this sem crashes the device)
        wsem = nc.gpsimd.wait_ge(dma_sem, 16)
        add_dep_helper(wsem.ins, trig.ins, sync=False, reason="wait after fire")

    fn = nc.m.functions[0]
    ET = mybir.EngineType

    # Same-engine proc-sem waits on ENGINE-executed compute ops are
    # redundant (the engine runs its queue in order) but cost ~SEM_DELAY
    # when the predecessor has only just finished; strip them.  Sequencer-
    # only instructions (triggers, event sems, waits) genuinely need them —
    # the SEQ runs ahead of the engine — so only compute ops are touched.
    _eng_prefix = {
        ET.Pool: "Pool_",
        ET.Activation: "Activation_",
        ET.DVE: "DVE_",
        ET.PE: "PE_",
    }
    _strippable = {
        "InstActivation",
        "InstTensorScalarPtr",
        "InstTensorReduce",
        "InstReciprocal",
        "InstMatmult",
        "InstMemset",
    }
    for block in fn.blocks:
        for ins in block.instructions:
            si = ins.sync_info
            pref = _eng_prefix.get(ins.engine)
            if (
                si is None
                or not si.on_wait
                or pref is None
                or type(ins).__name__ not in _strippable
            ):
                continue
            kept_w = [
                w
                for w in si.on_wait
                if not (w.ant_name and w.ant_name.startswith(pref))
            ]
            if len(kept_w) != len(si.on_wait):
                si.on_wait = kept_w

    # Tile's epilogue waits on the prep's DMASW proc sem, which only the real
    # SWDGE hardware auto-bumps — drop them (kernel end is still gated on the
    # true DMA-completion sem via the Pool queue).
    for block in fn.blocks:
        for ins in block.instructions:
            si = ins.sync_info
            if si is None or not si.on_wait:
                continue
            if any(w.ant_name and w.ant_name.startswith("DMASW") for w in si.on_wait):
                si.on_wait = [
                    w
                    for w in si.on_wait
                    if not (w.ant_name and w.ant_name.startswith("DMASW"))
                ]

    # Fold standalone Pool event-sem waits into the trigger instruction
    for block in fn.blocks:
        insts = list(block.instructions)
        for idx, ins in enumerate(insts):
            if type(ins).__name__ != "InstTriggerDma" or ins.sync_info is None:
                continue
            j = idx - 1
            while j >= 0:
                p = insts[j]
                psi = p.sync_info
                if (
                    type(p).__name__ == "InstEventSemaphore"
                    and p.engine == ET.Pool
                    and psi is not None
                    and not psi.on_update
                    and psi.on_wait
                ):
                    ins.sync_info.on_wait = list(psi.on_wait) + list(
                        ins.sync_info.on_wait
                    )
                    psi.on_wait = []
                    j -= 1
                else:
                    break

    # ---- delete the framework const-AP memsets from block 0 (they gate the
    # preamble barrier behind ~440ns of Pool time).  Safety: only delete if
    # no other instruction references a const-* tensor.
    def _memrefs(ins):
        refs = []
        for o in list(getattr(ins, "outs", [])) + list(getattr(ins, "ins", [])):
            r = getattr(o, "memref", None)
            if r:
                refs.append(r)
        return refs

    used_consts = set()
    b0_const_memsets = []
    for bi, block in enumerate(fn.blocks):
        for ins in block.instructions:
            refs = [r for r in _memrefs(ins) if r.startswith("const-")]
            if not refs:
                continue
            if bi == 0 and type(ins).__name__ == "InstMemset":
                b0_const_memsets.append(ins)
            else:
                used_consts.update(refs)
    dead = [
        i for i in b0_const_memsets if not (set(_memrefs(i)) & used_consts)
    ]
    fn.blocks[0].instructions = [
        i for i in fn.blocks[0].instructions if i not in dead
    ]

    # ---- move the input DMACopy into block 0, before SP's preamble drain:
    # its 650ns SEQ dispatch + 625ns HWDGE generation + 650ns DGE delay then
    # run from t=0 instead of after the barrier.
    b0, b1 = fn.blocks[0], fn.blocks[1]
    dma_ins = None
    for ins in b1.instructions:
        if type(ins).__name__ == "InstDMACopy" and ins.engine == ET.SP:
            dma_ins = ins
            break
    if dma_ins is not None and (dma_ins.sync_info is None or not dma_ins.sync_info.on_wait):
        b1.instructions = [i for i in b1.instructions if i is not dma_ins]
        sp_drain_idx = next(
            i
            for i, ins in enumerate(b0.instructions)
            if type(ins).__name__ == "InstDrain" and ins.engine == ET.SP
        )
        b0.instructions = (
            b0.instructions[: sp_drain_idx + 1]
            + [dma_ins]
            + b0.instructions[sp_drain_idx + 1 :]
        )

    # ---- strip the epilogue's two cross-engine barrier rounds (block 2):
    # delete the barrier event-sems and un-wire the drains' barrier waits and
    # gather bumps.  Kernel end stays gated on kv_dma>=16 via the Pool queue.
    b2 = fn.blocks[2]
    clear = next(
        (
            i
            for i in b2.instructions
            if type(i).__name__ == "InstISA"
            and getattr(i, "op_name", "") == "EVENT_SEMAPHORE_RANGE_CLEAR"
        ),
        None,
    )
    if clear is not None:
        rng = clear.ant_dict
        kv_id = 155
        for blk in fn.blocks:
            for i in blk.instructions:
                si = i.sync_info
                if si:
                    for x in list(si.on_wait or []) + list(si.on_update or []):
                        if x.ant_name == "kv_dma":
                            kv_id = x.id
        if not (rng["range_first"] <= kv_id <= rng["range_last"]):
            b2.instructions = [i for i in b2.instructions if i is not clear]
            b1i = fn.blocks[1].instructions
            widx = next(
                i
                for i, ins in enumerate(b1i)
                if type(ins).__name__ == "InstISA"
                and ins.engine == ET.Pool
                and any(
                    x.ant_name == "kv_dma" for x in (ins.sync_info.on_wait or [])
                )
                if ins.sync_info
            ) if False else None
            # place right before the Pool kv wait (an InstISA wait_ge)
            for i, ins in enumerate(b1i):
                si = ins.sync_info
                if (
                    ins.engine == ET.Pool
                    and si is not None
                    and any(x.ant_name == "kv_dma" for x in (si.on_wait or []))
                ):
                    b1i.insert(i, clear)
                    break
    pool_drains = [
        i
        for i in b2.instructions
        if type(i).__name__ == "InstDrain" and i.engine == ET.Pool
    ]
    kept = []
    for ins in b2.instructions:
        nm = getattr(ins, "name", "") or ""
        if type(ins).__name__ == "InstEventSemaphore" and nm.startswith("barrier_"):
            continue
        if ins in pool_drains:
            continue
        si = ins.sync_info
        if si is not None:
            if si.on_wait:
                si.on_wait = [
                    w
                    for w in si.on_wait
                    if not (w.ant_name and w.ant_name.startswith("barrier_"))
                ]
            if si.on_update:
                si.on_update = [
                    u
                    for u in si.on_update
                    if not (u.ant_name and u.ant_name.startswith("barrier_"))
                ]
        kept.append(ins)
    b2.instructions = kept

    nc.compile()
    _CACHE["nc"] = nc
    return nc


def kernel(pixel_coords, attn_weights, in_frame_mask, log_sigma, **kwargs):
    pc = np.asarray(pixel_coords, dtype=np.float64)  # (B, N, 2)
    aw = np.asarray(attn_weights, dtype=np.float64)
    mf = np.asarray(in_frame_mask).astype(np.float64)
    ls = float(np.asarray(log_sigma, dtype=np.float32))

    sig2 = np.exp(2.0 * ls)
    c = -0.5 / (sig2 + 1e-6)
    w = aw * mf
    lnw = np.log(np.maximum(w, 1e-20))  # clamp: exp(-46) == 0 for any pixel
    grid = np.arange(W, dtype=np.float64)

    nc = _build()
    from concourse.bass_utils import run_bass_kernel_spmd

    in_maps = []
    for i in range(NCORES):
        pkm = np.zeros((5, 2 * W2), dtype=np.float64)
        pkm[0, 0:W] = grid
        pkm[1, 0:W] = 1.0
        pkm[2, W:W2] = grid
        pkm[3, W:W2] = 1.0
        pkm[4, 0:W] = grid * grid
        pkm[4, W:W2] = grid * grid
        for b in range(BPC):
            s = BPC * i + b
            cs = slice(W2 + N * b, W2 + N * (b + 1))
            u = pc[s, :, 0]
            v = pc[s, :, 1]
            pkm[0, cs] = -2.0 * c * u
            pkm[1, cs] = c * u * u + lnw[s]
            pkm[2, cs] = -2.0 * c * v
            pkm[3, cs] = c * v * v
            pkm[4, cs] = c
        in_maps.append({"pkm": pkm.astype(np.float32)})
    res = run_bass_kernel_spmd(nc, in_maps, core_ids=list(range(NCORES)))
    return np.concatenate(
        [r["out"].reshape(BPC, HP, W)[:, :H, :] for r in res.results], axis=0
    )


# revision 27
# speedup vs baseline: 1.0401x; 1.0011x over previous
"""Trainium2 Bass kernel for AudioAttentionMapGenerator.

Math (reference):
    sigma = exp(log_sigma); c = -0.5 / (sigma^2 + 1e-6)
    w_n   = attn_weights * mask
    map[b,h,w] = sum_n w_n * exp(c*((h-v_bn)^2 + (w-u_bn)^2))
    out = map / (max_hw(map) + 1e-6)

Key restructuring vs the previous STT-based kernel: the Gaussian exponent is
an affine function of the grid, so the host packs per-point coefficient rows
    a_u = -2c*u,  b_u = c*u^2 + ln(w),  a_v = -2c*v,  b_v = c*v^2,  c
and a constant grid matrix M = [grid|0, 1|0, 0|grid, 0|1, grid2|grid2].
One [10,448] DMA delivers both; a single f32r matmul per sample
(S^T [5x128] @ M [5x448]) produces the full exponent tile
[128 pts, 448 = x-block|y-block] in PSUM, and one Exp per sample turns it
into the weighted Gaussian factors (ln w folded into the x-block bias, so
the separable map matmul needs no extra weighting pass):
    map = Gy^T @ (w*Gx), two bf16 matmuls per sample (h chunked 128+96).

Per-sample normalization: stride-2-subsampled free-dim reduce_max (DVE; the
map's curvature scale is sigma=20px so a 2px grid undersamples the peak by
<0.1%), partition all-reduce (GPSIMD), reciprocal (DVE), then the scale is
split h-chunk-wise across ACT and DVE so each sample's scale finishes in one
chunk-time and both engines' pieces overlap.

Latency plumbing (the bulk of the win):
  - The input DMACopy is moved (post-build IR surgery) into block 0 BEFORE
    the preamble barrier: its 650ns SP-SEQ dispatch, 625ns HWDGE generation
    and 650ns DGE->DMA delay all run from t=0 instead of after the barrier.
  - The framework's four const-AP memsets (unused by this kernel) are
    deleted from block 0; they otherwise hold the preamble barrier (and
    with it every engine's start) hostage for ~440ns behind Pool.
  - Output leaves via a kv_writeback prepared early and fired by
    trigger_dma after the four scale pieces (staging buffer under two
    aliased SBUF names so the prep can run ~4us before the scales).
  - The Tile epilogue's two cross-engine barrier rounds after the final
    DMA-completion wait are stripped (the Pool queue still gates kernel end
    on kv_dma>=16; the barriers only re-synchronized already-idle engines).
  - PE warm-up matmuls tile the gaps so the exponent matmuls hit the mid
    p-state and the map matmuls hit the full-speed clock.
"""

import sys

import numpy as np

if "/opt/trn_rl_repo" not in sys.path:
    sys.path.insert(0, "/opt/trn_rl_repo")

B, N, H, W = 16, 128, 224, 224
NCORES = 8
BPC = B // NCORES  # samples per core
HP = 256  # padded rows per sample in DRAM (2 chunks of 128)
W2 = 2 * W  # 448: x-block | y-block

_CACHE = {}


def _build():
    if "nc" in _CACHE:
        return _CACHE["nc"]

    from contextlib import ExitStack

    import concourse.bass_isa as bass_isa
    import concourse.tile as tile
    from concourse import bacc, mybir
    from concourse.tile import add_dep_helper

    f32 = mybir.dt.float32
    f32r = mybir.dt.float32r
    bf16 = mybir.dt.bfloat16
    i32 = mybir.dt.int32
    AF = mybir.ActivationFunctionType
    AX = mybir.AxisListType

    nc = bacc.Bacc(
        "TRN2",
        target_bir_lowering=False,
        debug=False,
        enable_asserts=False,
        num_devices=NCORES,
    )
    # packed per-core input, one row-major DMA, all bf16 with hi/lo split
    # rows so the exponent matmul is exact to ~1e-3 in the exponent (bf16
    # products accumulate exactly in f32 PSUM; f32r would round to ~13-bit
    # mantissa on hardware).  Matmul operands must share base partition 0,
    # so M and S live side by side in the free dim of one [11, 704] tile:
    #   cols 0:448 = M rows [g|0, g|0, 1|0, 1|0, 0|g, 0|g, 0|1, 0|1,
    #                        g2h|g2h, g2l|g2l, g2h|g2h]
    #   cols 448:576 = S0 rows [ahu, alu, bhu, blu, ahv, alv, bhv, blv,
    #                           ch, ch, cl]   (a = -2c*u, b = c*u^2 + ln w)
    #   cols 576:704 = S1 rows.
    NR = 11
    pkm = nc.dram_tensor("pkm", (NR, W2 + 2 * N), bf16, kind="ExternalInput").ap()
    # per-sample rows padded 224->256 so every 128-row writeback chunk stays
    # in-bounds; host discards rows 224:256 of each sample
    out = nc.dram_tensor("out", (BPC * HP, W), f32, kind="ExternalOutput")

    with ExitStack() as ctx:
        tc = ctx.enter_context(tile.TileContext(nc))
        consts = ctx.enter_context(tc.tile_pool(name="consts", bufs=1))
        psum = ctx.enter_context(tc.tile_pool(name="psum", bufs=1, space="PSUM"))

        # ---- input DMA (relocated to block 0 pre-barrier by surgery below).
        pkt = consts.tile([NR, W2 + 2 * N], bf16)
        nc.sync.dma_start(out=pkt, in_=pkm)

        # ---- output writeback descriptors prepared NOW, fired at the end.
        # out viewed as [batch=4 chunks, 128 rows, dho=1, 224]; staging is
        # [128 partitions, 4 chunks, 224] (chunk c partition p = row 128c+p).
        # Two aliased names for the same SBUF bytes: scales write st_w, the
        # prep's descriptors read st_r (Tile would otherwise WAR-pin the
        # scales to the in-flight DMA); real ordering is via trigger deps.
        st_w = nc.alloc_sbuf_tensor("st_w", (128, 2 * BPC, W), f32)
        st_r = nc.alloc_sbuf_tensor_at(
            "st_r", (128, 2 * BPC, W), f32, offset=nc.lookup_mloc(st_w).addr
        )
        st = st_w.ap()
        ctxi = consts.tile([128, 2 * BPC], i32)
        nc.gpsimd.memset(ctxi, 0)
        dma_sem = nc.alloc_semaphore("kv_dma")
        out4d = out.ap().rearrange("(c p) (d w) -> c p d w", p=128, d=1)
        i_kvprep = nc.gpsimd.kv_writeback(
            out4d,
            st_r.ap().rearrange("p (d b) w -> p d b w", d=1),
            ctxi[:, :],
            prepare_only=True,
            sem=dma_sem,
        )


        # Sample processing order is [1, 0]: sample1's exponent, Exp, coarse
        # map and full normalization chain all complete while sample0's Exp
        # is still running, so the two samples' reduce chains never contend.
        # PSUM ownership per engine (an ACT activation reading PSUM consumes
        # the accumulator, serializing any other toucher of that tile):
        #   ACT: eps0, eps1 (Exps), pmap1 (scale_s1), pm0c1 (scale_s0c1)
        #   DVE: cm0, cm1 (reduces), pm0c0 (scale_s0c0)
        eps, exps = [], []
        for b in range(BPC):
            ep = psum.tile([128, W2], f32, tag=f"eps{b}", name=f"eps{b}")
            eps.append(ep)
            exps.append(consts.tile([128, W2], bf16, tag=f"exp{b}", name=f"exp{b}"))
        pmap1 = psum.tile([128, 2, W], f32, tag="pmap1", name="pmap1")
        nc.vector.memset(pmap1[96:128, 1, :], 0.0)
        pmap0 = psum.tile([128, 2, W], f32, tag="pmap0", name="pmap0")
        nc.vector.memset(pmap0[96:128, 1, :], 0.0)

        # ---- exponent matmuls: ep[n, x|y] = S_b^T @ M  (bf16, 1 cyc/col) ----
        mov = pkt[0:NR, 0:W2]
        i_mmexp = {}
        for b in (1, 0):
            stat = pkt[0:NR, W2 + b * N : W2 + (b + 1) * N]
            i_mmexp[b] = nc.tensor.matmul(eps[b][:, :], stat, mov)

        # ---- Exps: one per sample, PSUM -> bf16 SBUF.  Sample0 (the late
        # chain) additionally gets a small coarse Exp over a stride-3
        # subsample of its exponent, emitted BEFORE its full Exp: its coarse
        # map and max chain start ~250ns earlier, while its full-map matmuls
        # (which have ~500ns of slack against the max chain) absorb the
        # 310ns Exp delay.
        CW = (W + 2) // 3  # 75
        i_exp = {}
        i_exp[1] = nc.scalar.activation(exps[1], eps[1][:, :], AF.Exp)
        exp0c = consts.tile([128, 2 * CW], bf16)
        i_exp0c = nc.scalar.activation(exp0c, eps[0][:, 0 : W2 : 3], AF.Exp)
        i_exp[0] = nc.scalar.activation(exps[0], eps[0][:, :], AF.Exp)

        # coarse maps for the max: stride-3 subsample in BOTH dims -> [75,75]
        # (peak deficit <= c*(1.5^2*2) ~ 0.56% relative, uniform)
        cms, i_cms = {}, {}
        cp1 = psum.tile([128, CW], f32, tag="cmap1", name="cmap1")
        i_cms[1] = nc.tensor.matmul(
            cp1[0:CW, :], exps[1][:, W : W2 : 3], exps[1][:, 0 : W : 3]
        )
        cms[1] = cp1
        cp0 = psum.tile([128, CW], f32, tag="cmap0", name="cmap0")
        i_cms[0] = nc.tensor.matmul(
            cp0[0:CW, :], exp0c[:, CW : 2 * CW], exp0c[:, 0:CW]
        )
        cms[0] = cp0

        # ---- map matmuls (bf16): chunks of 128/96 h-rows ----
        i_maps = [
            nc.tensor.matmul(pmap1[:, 0, :], exps[1][:, W : W + 128], exps[1][:, 0:W]),
            nc.tensor.matmul(pmap1[0:96, 1, :], exps[1][:, W + 128 : W2], exps[1][:, 0:W]),
            nc.tensor.matmul(pmap0[:, 0, :], exps[0][:, W : W + 128], exps[0][:, 0:W]),
            nc.tensor.matmul(pmap0[0:96, 1, :], exps[0][:, W + 128 : W2], exps[0][:, 0:W]),
        ]

        # ---- PE queue order ----
        peq = [
            i_mmexp[1],
            i_mmexp[0],
            i_cms[1],
            i_maps[0],
            i_maps[1],
            i_cms[0],
            i_maps[2],
            i_maps[3],
        ]
        for a, b_ in zip(peq[1:], peq[:-1]):
            add_dep_helper(a.ins, b_.ins, sync=False, reason="pe order")

        # ---- per-sample normalization (max from the coarse maps) ----
        mrows, malls, rss = {}, {}, {}
        i_red, i_rs = {}, {}
        for b in (1, 0):
            mrow = consts.tile([128, 1], f32, tag=f"mrow{b}", name=f"mrow{b}")
            nc.vector.memset(mrow[64:128, :], 0.0)
            i_red[b] = nc.vector.reduce_max(mrow[0:CW, :], cms[b][0:CW, :], axis=AX.XY)
            mrows[b] = mrow
        for b in (1, 0):
            mall = consts.tile([128, 1], f32, tag=f"mall{b}", name=f"mall{b}")
            nc.gpsimd.partition_all_reduce(
                mall, mrows[b], channels=128, reduce_op=bass_isa.ReduceOp.max
            )
            malls[b] = mall
        for b in (1, 0):
            rs = consts.tile([128, 1], f32, tag=f"rs{b}", name=f"rs{b}")
            i_rs[b] = nc.vector.reciprocal(rs, malls[b])
            rss[b] = rs
        # scales: sample1 (early chain) on DVE right after its reciprocal;
        # sample0 (late, maps-gated) on ACT which is free once Exp0 retires
        i_scales = [
            nc.vector.tensor_scalar_mul(st[:, 2:4, :], pmap1[:, :, :], rss[1][:, 0:1]),
            nc.scalar.mul(st[:, 0:2, :], pmap0[:, :, :], rss[0][:, 0:1]),
        ]

        # DVE queue order: r1, r0, rs1, rs0, scale_s1
        dveq = [i_red[1], i_red[0], i_rs[1], i_rs[0], i_scales[0]]
        for a, b_ in zip(dveq[1:], dveq[:-1]):
            add_dep_helper(a.ins, b_.ins, sync=False, reason="dve order")
        # ACT queue order: exp1, exp0c, exp0, scale_s0
        actq = [i_exp[1], i_exp0c, i_exp[0], i_scales[1]]
        for a, b_ in zip(actq[1:], actq[:-1]):
            add_dep_helper(a.ins, b_.ins, sync=False, reason="act order")

        # ---- fire the prepared writeback ----
        trig = nc.gpsimd.trigger_dma(count=None)
        for s in i_scales:
            add_dep_helper(trig.ins, s.ins, sync=True, reason="st written")
        # pin the completion wait AFTER the trigger in the in-order Pool queue
        # (an SP-side wait on this sem crashes the device)
        wsem = nc.gpsimd.wait_ge(dma_sem, 16)
        add_dep_helper(wsem.ins, trig.ins, sync=False, reason="wait after fire")

    fn = nc.m.functions[0]
    ET = mybir.EngineType

    # Same-engine proc-sem waits on ENGINE-executed compute ops are
    # redundant (the engine runs its queue in order) but cost ~SEM_DELAY
    # when the predecessor has only just finished; strip them.  Sequencer-
    # only instructions (triggers, event sems, waits) genuinely need them —
    # the SEQ runs ahead of the engine — so only compute ops are touched.
    _eng_prefix = {
        ET.Pool: "Pool_",
        ET.Activation: "Activation_",
        ET.DVE: "DVE_",
        ET.PE: "PE_",
    }
    _strippable = {
        "InstActivation",
        "InstTensorScalarPtr",
        "InstTensorReduce",
        "InstReciprocal",
        "InstMatmult",
        "InstMemset",
    }
    for block in fn.blocks:
        for ins in block.instructions:
            si = ins.sync_info
            pref = _eng_prefix.get(ins.engine)
            if (
                si is None
                or not si.on_wait
                or pref is None
                or type(ins).__name__ not in _strippable
            ):
                continue
            kept_w = [
                w
                for w in si.on_wait
                if not (w.ant_name and w.ant_name.startswith(pref))
            ]
            if len(kept_w) != len(si.on_wait):
                si.on_wait = kept_w

    # Tile's epilogue waits on the prep's DMASW proc sem, which only the real
    # SWDGE hardware auto-bumps — drop them (kernel end is still gated on the
    # true DMA-completion sem via the Pool queue).
    for block in fn.blocks:
        for ins in block.instructions:
            si = ins.sync_info
            if si is None or not si.on_wait:
                continue
            if any(w.ant_name and w.ant_name.startswith("DMASW") for w in si.on_wait):
                si.on_wait = [
                    w
                    for w in si.on_wait
                    if not (w.ant_name and w.ant_name.startswith("DMASW"))
                ]

    # Fold standalone Pool event-sem waits into the trigger instruction
    for block in fn.blocks:
        insts = list(block.instructions)
        for idx, ins in enumerate(insts):
            if type(ins).__name__ != "InstTriggerDma" or ins.sync_info is None:
                continue
            j = idx - 1
            while j >= 0:
                p = insts[j]
                psi = p.sync_info
                if (
                    type(p).__name__ == "InstEventSemaphore"
                    and p.engine == ET.Pool
                    and psi is not None
                    and not psi.on_update
                    and psi.on_wait
                ):
                    ins.sync_info.on_wait = list(psi.on_wait) + list(
                        ins.sync_info.on_wait
                    )
                    psi.on_wait = []
                    j -= 1
                else:
                    break

    # ---- delete the framework const-AP memsets from block 0 (they gate the
    # preamble barrier behind ~440ns of Pool time).  Safety: only delete if
    # no other instruction references a const-* tensor.
    def _memrefs(ins):
        refs = []
        for o in list(getattr(ins, "outs", [])) + list(getattr(ins, "ins", [])):
            r = getattr(o, "memref", None)
            if r:
                refs.append(r)
        return refs

    used_consts = set()
    b0_const_memsets = []
    for bi, block in enumerate(fn.blocks):
        for ins in block.instructions:
            refs = [r for r in _memrefs(ins) if r.startswith("const-")]
            if not refs:
                continue
            if bi == 0 and type(ins).__name__ == "InstMemset":
                b0_const_memsets.append(ins)
            else:
                used_consts.update(refs)
    dead = [
        i for i in b0_const_memsets if not (set(_memrefs(i)) & used_consts)
    ]
    fn.blocks[0].instructions = [
        i for i in fn.blocks[0].instructions if i not in dead
    ]

    # ---- move the input DMACopy into block 0, before SP's preamble drain:
    # its 650ns SEQ dispatch + 625ns HWDGE generation + 650ns DGE delay then
    # run from t=0 instead of after the barrier.
    b0, b1 = fn.blocks[0], fn.blocks[1]
    dma_ins = None
    for ins in b1.instructions:
        if type(ins).__name__ == "InstDMACopy" and ins.engine == ET.SP:
            dma_ins = ins
            break
    if dma_ins is not None and (dma_ins.sync_info is None or not dma_ins.sync_info.on_wait):
        b1.instructions = [i for i in b1.instructions if i is not dma_ins]
        sp_drain_idx = next(
            i
            for i, ins in enumerate(b0.instructions)
            if type(ins).__name__ == "InstDrain" and ins.engine == ET.SP
        )
        b0.instructions = (
            b0.instructions[: sp_drain_idx + 1]
            + [dma_ins]
            + b0.instructions[sp_drain_idx + 1 :]
        )

    # ---- strip the epilogue's two cross-engine barrier rounds (block 2):
    # delete the barrier event-sems and un-wire the drains' barrier waits and
    # gather bumps.  Kernel end stays gated on kv_dma>=16 via the Pool queue.
    b2 = fn.blocks[2]
    clear = next(
        (
            i
            for i in b2.instructions
            if type(i).__name__ == "InstISA"
            and getattr(i, "op_name", "") == "EVENT_SEMAPHORE_RANGE_CLEAR"
        ),
        None,
    )
    if clear is not None:
        rng = clear.ant_dict
        kv_id = 155
        for blk in fn.blocks:
            for i in blk.instructions:
                si = i.sync_info
                if si:
                    for x in list(si.on_wait or []) + list(si.on_update or []):
                        if x.ant_name == "kv_dma":
                            kv_id = x.id
        if not (rng["range_first"] <= kv_id <= rng["range_last"]):
            b2.instructions = [i for i in b2.instructions if i is not clear]
            b1i = fn.blocks[1].instructions
            widx = next(
                i
                for i, ins in enumerate(b1i)
                if type(ins).__name__ == "InstISA"
                and ins.engine == ET.Pool
                and any(
                    x.ant_name == "kv_dma" for x in (ins.sync_info.on_wait or [])
                )
                if ins.sync_info
            ) if False else None
            # place right before the Pool kv wait (an InstISA wait_ge)
            for i, ins in enumerate(b1i):
                si = ins.sync_info
                if (
                    ins.engine == ET.Pool
                    and si is not None
                    and any(x.ant_name == "kv_dma" for x in (si.on_wait or []))
                ):
                    b1i.insert(i, clear)
                    break
    pool_drains = [
        i
        for i in b2.instructions
        if type(i).__name__ == "InstDrain" and i.engine == ET.Pool
    ]
    kept = []
    for ins in b2.instructions:
        nm = getattr(ins, "name", "") or ""
        if type(ins).__name__ == "InstEventSemaphore" and nm.startswith("barrier_"):
            continue
        if ins in pool_drains:
            continue
        si = ins.sync_info
        if si is not None:
            if si.on_wait:
                si.on_wait = [
                    w
                    for w in si.on_wait
                    if not (w.ant_name and w.ant_name.startswith("barrier_"))
                ]
            if si.on_update:
                si.on_update = [
                    u
                    for u in si.on_update
                    if not (u.ant_name and u.ant_name.startswith("barrier_"))
                ]
        kept.append(ins)
    b2.instructions = kept

    nc.compile()
    _CACHE["nc"] = nc
    return nc


def kernel(pixel_coords, attn_weights, in_frame_mask, log_sigma, **kwargs):
    pc = np.asarray(pixel_coords, dtype=np.float64)  # (B, N, 2)
    aw = np.asarray(attn_weights, dtype=np.float64)
    mf = np.asarray(in_frame_mask).astype(np.float64)
    ls = float(np.asarray(log_sigma, dtype=np.float32))

    sig2 = np.exp(2.0 * ls)
    c = -0.5 / (sig2 + 1e-6)
    w = aw * mf
    lnw = np.log(np.maximum(w, 1e-20))  # clamp: exp(-46) == 0 for any pixel
    grid = np.arange(W, dtype=np.float64)

    nc = _build()
    from concourse.bass_utils import run_bass_kernel_spmd
    import ml_dtypes

    bf = ml_dtypes.bfloat16

    def split(x):
        hi = np.asarray(x, np.float64).astype(bf).astype(np.float64)
        lo = (np.asarray(x, np.float64) - hi).astype(bf).astype(np.float64)
        return hi, lo

    g2h, g2l = split(grid * grid)
    ch, cl = split(c)
    in_maps = []
    for i in range(NCORES):
        pkm = np.zeros((11, 2 * W + 2 * N), dtype=np.float64)
        for r in (0, 1):
            pkm[r, 0:W] = grid
        pkm[2, 0:W] = 1.0
        pkm[3, 0:W] = 1.0
        for r in (4, 5):
            pkm[r, W : 2 * W] = grid
        pkm[6, W : 2 * W] = 1.0
        pkm[7, W : 2 * W] = 1.0
        pkm[8, 0:W] = g2h
        pkm[8, W : 2 * W] = g2h
        pkm[9, 0:W] = g2l
        pkm[9, W : 2 * W] = g2l
        pkm[10, 0:W] = g2h
        pkm[10, W : 2 * W] = g2h
        for b in range(BPC):
            s = BPC * i + b
            cs = slice(2 * W + N * b, 2 * W + N * (b + 1))
            u = pc[s, :, 0]
            v = pc[s, :, 1]
            pkm[0, cs], pkm[1, cs] = split(-2.0 * c * u)
            pkm[2, cs], pkm[3, cs] = split(c * u * u + lnw[s])
            pkm[4, cs], pkm[5, cs] = split(-2.0 * c * v)
            pkm[6, cs], pkm[7, cs] = split(c * v * v)
            pkm[8, cs] = ch
            pkm[9, cs] = ch
            pkm[10, cs] = cl
        in_maps.append({"pkm": pkm.astype(bf)})
    res = run_bass_kernel_spmd(nc, in_maps, core_ids=list(range(NCORES)))
    return np.concatenate(
        [r["out"].reshape(BPC, HP, W)[:, :H, :] for r in res.results], axis=0
    )


# revision 29
# speedup vs baseline: 1.0452x; 1.0049x over previous
"""Trainium2 Bass kernel for AudioAttentionMapGenerator.

Math (reference):
    sigma = exp(log_sigma); c = -0.5 / (sigma^2 + 1e-6)
    w_n   = attn_weights * mask
    map[b,h,w] = sum_n w_n * exp(c*((h-v_bn)^2 + (w-u_bn)^2))
    out = map / (max_hw(map) + 1e-6)

Key restructuring vs the previous STT-based kernel: the Gaussian exponent is
an affine function of the grid, so the host packs per-point coefficient rows
    a_u = -2c*u,  b_u = c*u^2 + ln(w),  a_v = -2c*v,  b_v = c*v^2,  c
and a constant grid matrix M = [grid|0, 1|0, 0|grid, 0|1, grid2|grid2].
One [10,448] DMA delivers both; a single f32r matmul per sample
(S^T [5x128] @ M [5x448]) produces the full exponent tile
[128 pts, 448 = x-block|y-block] in PSUM, and one Exp per sample turns it
into the weighted Gaussian factors (ln w folded into the x-block bias, so
the separable map matmul needs no extra weighting pass):
    map = Gy^T @ (w*Gx), two bf16 matmuls per sample (h chunked 128+96).

Per-sample normalization: stride-2-subsampled free-dim reduce_max (DVE; the
map's curvature scale is sigma=20px so a 2px grid undersamples the peak by
<0.1%), partition all-reduce (GPSIMD), reciprocal (DVE), then the scale is
split h-chunk-wise across ACT and DVE so each sample's scale finishes in one
chunk-time and both engines' pieces overlap.

Latency plumbing (the bulk of the win):
  - The input DMACopy is moved (post-build IR surgery) into block 0 BEFORE
    the preamble barrier: its 650ns SP-SEQ dispatch, 625ns HWDGE generation
    and 650ns DGE->DMA delay all run from t=0 instead of after the barrier.
  - The framework's four const-AP memsets (unused by this kernel) are
    deleted from block 0; they otherwise hold the preamble barrier (and
    with it every engine's start) hostage for ~440ns behind Pool.
  - Output leaves via a kv_writeback prepared early and fired by
    trigger_dma after the four scale pieces (staging buffer under two
    aliased SBUF names so the prep can run ~4us before the scales).
  - The Tile epilogue's two cross-engine barrier rounds after the final
    DMA-completion wait are stripped (the Pool queue still gates kernel end
    on kv_dma>=16; the barriers only re-synchronized already-idle engines).
  - PE warm-up matmuls tile the gaps so the exponent matmuls hit the mid
    p-state and the map matmuls hit the full-speed clock.
"""

import sys

import numpy as np

if "/opt/trn_rl_repo" not in sys.path:
    sys.path.insert(0, "/opt/trn_rl_repo")

B, N, H, W = 16, 128, 224, 224
NCORES = 8
BPC = B // NCORES  # samples per core
HP = 256  # padded rows per sample in DRAM (2 chunks of 128)
W2 = 2 * W  # 448: x-block | y-block

_CACHE = {}


def _build():
    if "nc" in _CACHE:
        return _CACHE["nc"]

    from contextlib import ExitStack

    import concourse.bass_isa as bass_isa
    import concourse.tile as tile
    from concourse import bacc, mybir
    from concourse.tile import add_dep_helper

    f32 = mybir.dt.float32
    f32r = mybir.dt.float32r
    bf16 = mybir.dt.bfloat16
    i32 = mybir.dt.int32
    AF = mybir.ActivationFunctionType
    AX = mybir.AxisListType

    nc = bacc.Bacc(
        "TRN2",
        target_bir_lowering=False,
        debug=False,
        enable_asserts=False,
        num_devices=NCORES,
    )
    # packed per-core input, one row-major DMA, all bf16 with hi/lo split
    # rows so the exponent matmul is exact to ~1e-3 in the exponent (bf16
    # products accumulate exactly in f32 PSUM; f32r would round to ~13-bit
    # mantissa on hardware).  Matmul operands must share base partition 0,
    # so M and S live side by side in the free dim of one [11, 704] tile:
    #   cols 0:448 = M rows [g|0, g|0, 1|0, 1|0, 0|g, 0|g, 0|1, 0|1,
    #                        g2h|g2h, g2l|g2l, g2h|g2h]
    #   cols 448:576 = S0 rows [ahu, alu, bhu, blu, ahv, alv, bhv, blv,
    #                           ch, ch, cl]   (a = -2c*u, b = c*u^2 + ln w)
    #   cols 576:704 = S1 rows.
    NR = 11
    pkm = nc.dram_tensor("pkm", (NR, W2 + 2 * N), bf16, kind="ExternalInput").ap()
    # per-sample rows padded 224->256 so every 128-row writeback chunk stays
    # in-bounds; host discards rows 224:256 of each sample
    out = nc.dram_tensor("out", (BPC * HP, W), f32, kind="ExternalOutput")

    with ExitStack() as ctx:
        tc = ctx.enter_context(tile.TileContext(nc))
        consts = ctx.enter_context(tc.tile_pool(name="consts", bufs=1))
        psum = ctx.enter_context(tc.tile_pool(name="psum", bufs=1, space="PSUM"))

        # ---- input DMA (relocated to block 0 pre-barrier by surgery below).
        pkt = consts.tile([NR, W2 + 2 * N], bf16)
        nc.sync.dma_start(out=pkt, in_=pkm)

        # ---- output writeback descriptors prepared NOW, fired at the end.
        # out viewed as [batch=4 chunks, 128 rows, dho=1, 224]; staging is
        # [128 partitions, 4 chunks, 224] (chunk c partition p = row 128c+p).
        # Two aliased names for the same SBUF bytes: scales write st_w, the
        # prep's descriptors read st_r (Tile would otherwise WAR-pin the
        # scales to the in-flight DMA); real ordering is via trigger deps.
        st_w = nc.alloc_sbuf_tensor("st_w", (128, 2 * BPC, W), f32)
        st_r = nc.alloc_sbuf_tensor_at(
            "st_r", (128, 2 * BPC, W), f32, offset=nc.lookup_mloc(st_w).addr
        )
        st = st_w.ap()
        ctxi = consts.tile([128, 2 * BPC], i32)
        nc.gpsimd.memset(ctxi, 0)
        dma_sem = nc.alloc_semaphore("kv_dma")
        out4d = out.ap().rearrange("(c p) (d w) -> c p d w", p=128, d=1)
        i_kvprep = nc.gpsimd.kv_writeback(
            out4d,
            st_r.ap().rearrange("p (d b) w -> p d b w", d=1),
            ctxi[:, :],
            prepare_only=True,
            sem=dma_sem,
        )


        # Sample processing order is [1, 0]: sample1's exponent, Exp, coarse
        # map and full normalization chain all complete while sample0's Exp
        # is still running, so the two samples' reduce chains never contend.
        # PSUM ownership per engine (an ACT activation reading PSUM consumes
        # the accumulator, serializing any other toucher of that tile):
        #   ACT: eps0, eps1 (Exps), pmap1 (scale_s1), pm0c1 (scale_s0c1)
        #   DVE: cm0, cm1 (reduces), pm0c0 (scale_s0c0)
        eps, exps = [], []
        for b in range(BPC):
            ep = psum.tile([128, W2], f32, tag=f"eps{b}", name=f"eps{b}")
            eps.append(ep)
            exps.append(consts.tile([128, W2], bf16, tag=f"exp{b}", name=f"exp{b}"))
        pmap1 = psum.tile([128, 2, W], f32, tag="pmap1", name="pmap1")
        nc.vector.memset(pmap1[96:128, 1, :], 0.0)
        pmap0 = psum.tile([128, 2, W], f32, tag="pmap0", name="pmap0")
        nc.vector.memset(pmap0[96:128, 1, :], 0.0)

        # ---- exponent matmuls: ep[n, x|y] = S_b^T @ M  (bf16, 1 cyc/col) ----
        mov = pkt[0:NR, 0:W2]
        i_mmexp = {}
        for b in (1, 0):
            stat = pkt[0:NR, W2 + b * N : W2 + (b + 1) * N]
            i_mmexp[b] = nc.tensor.matmul(eps[b][:, :], stat, mov)

        # ---- Exps: one per sample, PSUM -> bf16 SBUF.  Sample0 (the late
        # chain) additionally gets a small coarse Exp over a stride-3
        # subsample of its exponent, emitted BEFORE its full Exp: its coarse
        # map and max chain start ~250ns earlier, while its full-map matmuls
        # (which have ~500ns of slack against the max chain) absorb the
        # 310ns Exp delay.
        CW = W // 4  # 56
        i_exp = {}
        i_exp[1] = nc.scalar.activation(exps[1], eps[1][:, :], AF.Exp)
        exp0c = consts.tile([128, 2 * CW], bf16)
        i_exp0c = nc.scalar.activation(exp0c, eps[0][:, 0 : W2 : 4], AF.Exp)
        i_exp[0] = nc.scalar.activation(exps[0], eps[0][:, :], AF.Exp)

        # coarse maps for the max: stride-4 subsample in BOTH dims -> [56,56]
        # (peak deficit <= c*(2^2*2) ~ 1% relative worst-case, uniform scale)
        cms, i_cms = {}, {}
        cp1 = psum.tile([128, CW], f32, tag="cmap1", name="cmap1")
        i_cms[1] = nc.tensor.matmul(
            cp1[0:CW, :], exps[1][:, W : W2 : 4], exps[1][:, 0 : W : 4]
        )
        cms[1] = cp1
        cp0 = psum.tile([128, CW], f32, tag="cmap0", name="cmap0")
        i_cms[0] = nc.tensor.matmul(
            cp0[0:CW, :], exp0c[:, CW : 2 * CW], exp0c[:, 0:CW]
        )
        cms[0] = cp0

        # ---- map matmuls (bf16): chunks of 128/96 h-rows ----
        i_maps = [
            nc.tensor.matmul(pmap1[:, 0, :], exps[1][:, W : W + 128], exps[1][:, 0:W]),
            nc.tensor.matmul(pmap1[0:96, 1, :], exps[1][:, W + 128 : W2], exps[1][:, 0:W]),
            nc.tensor.matmul(pmap0[:, 0, :], exps[0][:, W : W + 128], exps[0][:, 0:W]),
            nc.tensor.matmul(pmap0[0:96, 1, :], exps[0][:, W + 128 : W2], exps[0][:, 0:W]),
        ]

        # ---- PE queue order ----
        peq = [
            i_mmexp[1],
            i_mmexp[0],
            i_cms[1],
            i_maps[0],
            i_maps[1],
            i_cms[0],
            i_maps[2],
            i_maps[3],
        ]
        for a, b_ in zip(peq[1:], peq[:-1]):
            add_dep_helper(a.ins, b_.ins, sync=False, reason="pe order")

        # ---- per-sample normalization (max from the coarse maps) ----
        mrows, malls, rss = {}, {}, {}
        i_red, i_rs = {}, {}
        for b in (1, 0):
            mrow = consts.tile([128, 1], f32, tag=f"mrow{b}", name=f"mrow{b}")
            nc.vector.memset(mrow[:, :], 0.0)
            i_red[b] = nc.vector.reduce_max(mrow[0:CW, :], cms[b][0:CW, :], axis=AX.XY)
            mrows[b] = mrow
        for b in (1, 0):
            mall = consts.tile([128, 1], f32, tag=f"mall{b}", name=f"mall{b}")
            nc.gpsimd.partition_all_reduce(
                mall, mrows[b], channels=128, reduce_op=bass_isa.ReduceOp.max
            )
            malls[b] = mall
        for b in (1, 0):
            rs = consts.tile([128, 1], f32, tag=f"rs{b}", name=f"rs{b}")
            i_rs[b] = nc.vector.reciprocal(rs, malls[b])
            rss[b] = rs
        # scales: sample1 (early chain) on DVE right after its reciprocal;
        # sample0 (late, maps-gated) on ACT which is free once Exp0 retires
        i_scales = [
            nc.vector.tensor_scalar_mul(st[:, 2:4, :], pmap1[:, :, :], rss[1][:, 0:1]),
            nc.scalar.mul(st[:, 0:2, :], pmap0[:, :, :], rss[0][:, 0:1]),
        ]

        # DVE queue order: r1, r0, rs1, rs0, scale_s1
        dveq = [i_red[1], i_red[0], i_rs[1], i_rs[0], i_scales[0]]
        for a, b_ in zip(dveq[1:], dveq[:-1]):
            add_dep_helper(a.ins, b_.ins, sync=False, reason="dve order")
        # ACT queue order: exp1, exp0c, exp0, scale_s0
        actq = [i_exp[1], i_exp0c, i_exp[0], i_scales[1]]
        for a, b_ in zip(actq[1:], actq[:-1]):
            add_dep_helper(a.ins, b_.ins, sync=False, reason="act order")

        # ---- fire the prepared writeback ----
        trig = nc.gpsimd.trigger_dma(count=None)
        for s in i_scales:
            add_dep_helper(trig.ins, s.ins, sync=True, reason="st written")
        # pin the completion wait AFTER the trigger in the in-order Pool queue
        # (an SP-side wait on this sem crashes the device)
        wsem = nc.gpsimd.wait_ge(dma_sem, 16)
        add_dep_helper(wsem.ins, trig.ins, sync=False, reason="wait after fire")

    fn = nc.m.functions[0]
    ET = mybir.EngineType

    # Same-engine proc-sem waits on ENGINE-executed compute ops are
    # redundant (the engine runs its queue in order) but cost ~SEM_DELAY
    # when the predecessor has only just finished; strip them.  Sequencer-
    # only instructions (triggers, event sems, waits) genuinely need them —
    # the SEQ runs ahead of the engine — so only compute ops are touched.
    _eng_prefix = {
        ET.Pool: "Pool_",
        ET.Activation: "Activation_",
        ET.DVE: "DVE_",
        ET.PE: "PE_",
    }
    _strippable = {
        "InstActivation",
        "InstTensorScalarPtr",
        "InstTensorReduce",
        "InstReciprocal",
        "InstMatmult",
        "InstMemset",
    }
    for block in fn.blocks:
        for ins in block.instructions:
            si = ins.sync_info
            pref = _eng_prefix.get(ins.engine)
            if (
                si is None
                or not si.on_wait
                or pref is None
                or type(ins).__name__ not in _strippable
            ):
                continue
            kept_w = [
                w
                for w in si.on_wait
                if not (w.ant_name and w.ant_name.startswith(pref))
            ]
            if len(kept_w) != len(si.on_wait):
                si.on_wait = kept_w

    # Tile's epilogue waits on the prep's DMASW proc sem, which only the real
    # SWDGE hardware auto-bumps — drop them (kernel end is still gated on the
    # true DMA-completion sem via the Pool queue).
    for block in fn.blocks:
        for ins in block.instructions:
            si = ins.sync_info
            if si is None or not si.on_wait:
                continue
            if any(w.ant_name and w.ant_name.startswith("DMASW") for w in si.on_wait):
                si.on_wait = [
                    w
                    for w in si.on_wait
                    if not (w.ant_name and w.ant_name.startswith("DMASW"))
                ]

    # Fold standalone Pool event-sem waits into the trigger instruction
    for block in fn.blocks:
        insts = list(block.instructions)
        for idx, ins in enumerate(insts):
            if type(ins).__name__ != "InstTriggerDma" or ins.sync_info is None:
                continue
            j = idx - 1
            while j >= 0:
                p = insts[j]
                psi = p.sync_info
                if (
                    type(p).__name__ == "InstEventSemaphore"
                    and p.engine == ET.Pool
                    and psi is not None
                    and not psi.on_update
                    and psi.on_wait
                ):
                    ins.sync_info.on_wait = list(psi.on_wait) + list(
                        ins.sync_info.on_wait
                    )
                    psi.on_wait = []
                    j -= 1
                else:
                    break

    # ---- delete the framework const-AP memsets from block 0 (they gate the
    # preamble barrier behind ~440ns of Pool time).  Safety: only delete if
    # no other instruction references a const-* tensor.
    def _memrefs(ins):
        refs = []
        for o in list(getattr(ins, "outs", [])) + list(getattr(ins, "ins", [])):
            r = getattr(o, "memref", None)
            if r:
                refs.append(r)
        return refs

    used_consts = set()
    b0_const_memsets = []
    for bi, block in enumerate(fn.blocks):
        for ins in block.instructions:
            refs = [r for r in _memrefs(ins) if r.startswith("const-")]
            if not refs:
                continue
            if bi == 0 and type(ins).__name__ == "InstMemset":
                b0_const_memsets.append(ins)
            else:
                used_consts.update(refs)
    dead = [
        i for i in b0_const_memsets if not (set(_memrefs(i)) & used_consts)
    ]
    fn.blocks[0].instructions = [
        i for i in fn.blocks[0].instructions if i not in dead
    ]

    # ---- move the input DMACopy into block 0, before SP's preamble drain:
    # its 650ns SEQ dispatch + 625ns HWDGE generation + 650ns DGE delay then
    # run from t=0 instead of after the barrier.
    b0, b1 = fn.blocks[0], fn.blocks[1]
    dma_ins = None
    for ins in b1.instructions:
        if type(ins).__name__ == "InstDMACopy" and ins.engine == ET.SP:
            dma_ins = ins
            break
    if dma_ins is not None and (dma_ins.sync_info is None or not dma_ins.sync_info.on_wait):
        b1.instructions = [i for i in b1.instructions if i is not dma_ins]
        sp_drain_idx = next(
            i
            for i, ins in enumerate(b0.instructions)
            if type(ins).__name__ == "InstDrain" and ins.engine == ET.SP
        )
        b0.instructions = (
            b0.instructions[: sp_drain_idx + 1]
            + [dma_ins]
            + b0.instructions[sp_drain_idx + 1 :]
        )

    # ---- strip the epilogue's two cross-engine barrier rounds (block 2):
    # delete the barrier event-sems and un-wire the drains' barrier waits and
    # gather bumps.  Kernel end stays gated on kv_dma>=16 via the Pool queue.
    b2 = fn.blocks[2]
    clear = next(
        (
            i
            for i in b2.instructions
            if type(i).__name__ == "InstISA"
            and getattr(i, "op_name", "") == "EVENT_SEMAPHORE_RANGE_CLEAR"
        ),
        None,
    )
    if clear is not None:
        rng = clear.ant_dict
        kv_id = 155
        for blk in fn.blocks:
            for i in blk.instructions:
                si = i.sync_info
                if si:
                    for x in list(si.on_wait or []) + list(si.on_update or []):
                        if x.ant_name == "kv_dma":
                            kv_id = x.id
        if not (rng["range_first"] <= kv_id <= rng["range_last"]):
            b2.instructions = [i for i in b2.instructions if i is not clear]
            b1i = fn.blocks[1].instructions
            widx = next(
                i
                for i, ins in enumerate(b1i)
                if type(ins).__name__ == "InstISA"
                and ins.engine == ET.Pool
                and any(
                    x.ant_name == "kv_dma" for x in (ins.sync_info.on_wait or [])
                )
                if ins.sync_info
            ) if False else None
            # place right before the Pool kv wait (an InstISA wait_ge)
            for i, ins in enumerate(b1i):
                si = ins.sync_info
                if (
                    ins.engine == ET.Pool
                    and si is not None
                    and any(x.ant_name == "kv_dma" for x in (si.on_wait or []))
                ):
                    b1i.insert(i, clear)
                    break
    pool_drains = [
        i
        for i in b2.instructions
        if type(i).__name__ == "InstDrain" and i.engine == ET.Pool
    ]
    kept = []
    for ins in b2.instructions:
        nm = getattr(ins, "name", "") or ""
        if type(ins).__name__ == "InstEventSemaphore" and nm.startswith("barrier_"):
            continue
        if ins in pool_drains:
            continue
        si = ins.sync_info
        if si is not None:
            if si.on_wait:
                si.on_wait = [
                    w
                    for w in si.on_wait
                    if not (w.ant_name and w.ant_name.startswith("barrier_"))
                ]
            if si.on_update:
                si.on_update = [
                    u
                    for u in si.on_update
                    if not (u.ant_name and u.ant_name.startswith("barrier_"))
                ]
        kept.append(ins)
    b2.instructions = kept

    nc.compile()
    _CACHE["nc"] = nc
    return nc


def kernel(pixel_coords, attn_weights, in_frame_mask, log_sigma, **kwargs):
    pc = np.asarray(pixel_coords, dtype=np.float64)  # (B, N, 2)
    aw = np.asarray(attn_weights, dtype=np.float64)
    mf = np.asarray(in_frame_mask).astype(np.float64)
    ls = float(np.asarray(log_sigma, dtype=np.float32))

    sig2 = np.exp(2.0 * ls)
    c = -0.5 / (sig2 + 1e-6)
    w = aw * mf
    lnw = np.log(np.maximum(w, 1e-20))  # clamp: exp(-46) == 0 for any pixel
    grid = np.arange(W, dtype=np.float64)

    nc = _build()
    from concourse.bass_utils import run_bass_kernel_spmd
    import ml_dtypes

    bf = ml_dtypes.bfloat16

    def split(x):
        hi = np.asarray(x, np.float64).astype(bf).astype(np.float64)
        lo = (np.asarray(x, np.float64) - hi).astype(bf).astype(np.float64)
        return hi, lo

    g2h, g2l = split(grid * grid)
    ch, cl = split(c)
    in_maps = []
    for i in range(NCORES):
        pkm = np.zeros((11, 2 * W + 2 * N), dtype=np.float64)
        for r in (0, 1):
            pkm[r, 0:W] = grid
        pkm[2, 0:W] = 1.0
        pkm[3, 0:W] = 1.0
        for r in (4, 5):
            pkm[r, W : 2 * W] = grid
        pkm[6, W : 2 * W] = 1.0
        pkm[7, W : 2 * W] = 1.0
        pkm[8, 0:W] = g2h
        pkm[8, W : 2 * W] = g2h
        pkm[9, 0:W] = g2l
        pkm[9, W : 2 * W] = g2l
        pkm[10, 0:W] = g2h
        pkm[10, W : 2 * W] = g2h
        for b in range(BPC):
            s = BPC * i + b
            cs = slice(2 * W + N * b, 2 * W + N * (b + 1))
            u = pc[s, :, 0]
            v = pc[s, :, 1]
            pkm[0, cs], pkm[1, cs] = split(-2.0 * c * u)
            pkm[2, cs], pkm[3, cs] = split(c * u * u + lnw[s])
            pkm[4, cs], pkm[5, cs] = split(-2.0 * c * v)
            pkm[6, cs], pkm[7, cs] = split(c * v * v)
            pkm[8, cs] = ch
            pkm[9, cs] = ch
            pkm[10, cs] = cl
        in_maps.append({"pkm": pkm.astype(bf)})
    res = run_bass_kernel_spmd(nc, in_maps, core_ids=list(range(NCORES)))
    return np.concatenate(
        [r["out"].reshape(BPC, HP, W)[:, :H, :] for r in res.results], axis=0
    )
